# revision 1
# baseline (speedup 1.0000x reference)
"""Trainium2 Bass kernel for nn_AttentionBlock (B=8, LN=2048, IDM=HDM=ODM=1024).

Sharding: data-parallel over batch, one batch element per NeuronCore (8 cores).

Per-core computation (batch element b):
    queries = i @ q ; keys = i @ k                    [ln, hdm]
    scores  = queries @ keys.T                        [ln, ln]
    att     = softmax(scores, axis=-1)
    vls     = i @ v                                   [ln, idm]
    ret     = att @ vls + i
    out     = leaky_relu(ret @ mlp, 0.2) + bias

Precision strategy: the softmax exponent amplifies matmul operand
rounding, so the Q/K path (q/k projections and scores) uses 3-pass
bf16 split matmuls (hi/lo decomposition, ~fp32 quality). The value/MLP
path tolerates bf16. All accumulation is fp32 in PSUM.

Layout strategy: everything is computed with the contraction dim on
partitions. The host pre-transposes i (iT = i.T) and pre-splits
operands into bf16 hi/lo pairs; on-chip phases:
  A) kT/qT/vls projections (qT, vls staged via DRAM),
  B) per 128-row s-tile: scores -> softmax -> DMA-transpose(att) ->
     att @ vls (+residual) -> @ mlp -> leaky-relu + bias.
"""
import os
import numpy as np
import ml_dtypes

import concourse.bacc as bacc
import concourse.mybir as mybir
import concourse.tile as tile
from concourse import bass_utils

F32 = mybir.dt.float32
BF16 = mybir.dt.bfloat16
Act = mybir.ActivationFunctionType
Axis = mybir.AxisListType

LN = 2048      # sequence length
D = 1024       # idm = hdm = odm
N_CORES = 8
DC = D // 128      # 8 contraction chunks
ST = LN // 128     # 16 s-tiles
TB = LN // 512     # 4 t-blocks (N=512)
NEG_SLOPE = 0.2

_cached_nc = None


def _build():
    nc = bacc.Bacc("TRN2", target_bir_lowering=False, debug=False)

    # Inputs (per core): host provides iT (= i_b.T) and all weights as
    # bf16 hi/lo splits. bias stays fp32.
    iTh = nc.dram_tensor("iTh", [D, LN], BF16, kind="ExternalInput")
    iTl = nc.dram_tensor("iTl", [D, LN], BF16, kind="ExternalInput")
    qh = nc.dram_tensor("qh", [D, D], BF16, kind="ExternalInput")
    ql = nc.dram_tensor("ql", [D, D], BF16, kind="ExternalInput")
    kh = nc.dram_tensor("kh", [D, D], BF16, kind="ExternalInput")
    kl = nc.dram_tensor("kl", [D, D], BF16, kind="ExternalInput")
    vh = nc.dram_tensor("vh", [D, D], BF16, kind="ExternalInput")
    vl = nc.dram_tensor("vl", [D, D], BF16, kind="ExternalInput")
    mlpb = nc.dram_tensor("mlpb", [D, D], BF16, kind="ExternalInput")
    bias = nc.dram_tensor("bias", [LN, D], F32, kind="ExternalInput")
    out_d = nc.dram_tensor("out", [LN, D], F32, kind="ExternalOutput")

    # [D, X] viewed as [128 partitions, DC chunks, X]
    def pcv(t, x):
        return t.ap().rearrange("(c p) x -> p c x", p=128)

    iThv, iTlv = pcv(iTh, LN), pcv(iTl, LN)

    with tile.TileContext(nc) as tc:
        # --- persistent pool (lives through both phases) ---
        with tc.tile_pool(name="pers", bufs=1) as pers, \
             tc.tile_pool(name="dram", bufs=1, space="DRAM") as dram:
            kTh_sb = pers.tile([128, DC, LN], BF16)   # 32 KB/part
            kTl_sb = pers.tile([128, DC, LN], BF16)   # 32 KB/part
            alpha_ap = pers.tile([128, 1], F32)
            nc.vector.memset(alpha_ap, NEG_SLOPE)

            qTh_d = dram.tile([128, DC, LN], BF16)
            qTl_d = dram.tile([128, DC, LN], BF16)
            vls_d = dram.tile([128, ST, D], BF16)

            # ================= Phase A: projections =================
            with tc.tile_pool(name="pa_it", bufs=1) as pa_it, \
                 tc.tile_pool(name="pa_w", bufs=1) as pa_w, \
                 tc.tile_pool(name="pa_ev", bufs=4) as pa_ev, \
                 tc.tile_pool(name="pa_ps", bufs=4, space="PSUM") as pa_ps:
                ith_sb = pa_it.tile([128, DC, LN], BF16)
                itl_sb = pa_it.tile([128, DC, LN], BF16)
                nc.sync.dma_start(out=ith_sb, in_=iThv)
                nc.sync.dma_start(out=itl_sb, in_=iTlv)

                def load_w(hi_t, lo_t):
                    wh_sb = pa_w.tile([128, DC, D], BF16, name="wh_sb", tag="wh")
                    nc.sync.dma_start(out=wh_sb, in_=pcv(hi_t, D))
                    wl_sb = None
                    if lo_t is not None:
                        wl_sb = pa_w.tile([128, DC, D], BF16, name="wl_sb", tag="wl")
                        nc.sync.dma_start(out=wl_sb, in_=pcv(lo_t, D))
                    return wh_sb, wl_sb

                # --- qT[h, s] (3-pass split) -> DRAM hi/lo ---
                wh_sb, wl_sb = load_w(qh, ql)
                for hc in range(DC):
                    for sb in range(TB):
                        ps = pa_ps.tile([128, 512], F32, name="ps_prep", tag="prep")
                        s_sl = slice(sb * 512, sb * 512 + 512)
                        for dc in range(DC):
                            first = dc == 0
                            last = dc == DC - 1
                            lw = wh_sb[:, dc, hc * 128:hc * 128 + 128]
                            ll = wl_sb[:, dc, hc * 128:hc * 128 + 128]
                            nc.tensor.matmul(ps, lw, ith_sb[:, dc, s_sl], start=first, stop=False)
                            nc.tensor.matmul(ps, lw, itl_sb[:, dc, s_sl], start=False, stop=False)
                            nc.tensor.matmul(ps, ll, ith_sb[:, dc, s_sl], start=False, stop=last)
                        evh = pa_ev.tile([128, 512], BF16, name="evh", tag="evh")
                        evl = pa_ev.tile([128, 512], BF16, name="evl", tag="evl")
                        nc.vector.tensor_copy(evh, ps)
                        nc.vector.tensor_sub(evl, ps, evh)
                        nc.sync.dma_start(out=qTh_d[:, hc, s_sl], in_=evh)
                        nc.sync.dma_start(out=qTl_d[:, hc, s_sl], in_=evl)

                # --- vls[t, e] (2-pass: iT full x vh, iTh x vl) -> DRAM ---
                wh_sb, wl_sb = load_w(vh, vl)
                for tc_ in range(ST):
                    t_sl = slice(tc_ * 128, tc_ * 128 + 128)
                    for eb in range(2):
                        ps = pa_ps.tile([128, 512], F32, name="ps_prep2", tag="prep")
                        e_sl = slice(eb * 512, eb * 512 + 512)
                        for dc in range(DC):
                            first = dc == 0
                            last = dc == DC - 1
                            nc.tensor.matmul(ps, ith_sb[:, dc, t_sl], wh_sb[:, dc, e_sl], start=first, stop=False)
                            nc.tensor.matmul(ps, itl_sb[:, dc, t_sl], wh_sb[:, dc, e_sl], start=False, stop=False)
                            nc.tensor.matmul(ps, ith_sb[:, dc, t_sl], wl_sb[:, dc, e_sl], start=False, stop=last)
                        ev = pa_ev.tile([128, 512], BF16, name="ev_v", tag="evh")
                        nc.vector.tensor_copy(ev, ps)
                        nc.sync.dma_start(out=vls_d[:, tc_, e_sl], in_=ev)

                # --- kT[h, t] (3-pass split) -> resident SBUF hi/lo ---
                wh_sb, wl_sb = load_w(kh, kl)
                for hc in range(DC):
                    for tb in range(TB):
                        ps = pa_ps.tile([128, 512], F32, name="ps_prep3", tag="prep")
                        t_sl = slice(tb * 512, tb * 512 + 512)
                        for dc in range(DC):
                            first = dc == 0
                            last = dc == DC - 1
                            lw = wh_sb[:, dc, hc * 128:hc * 128 + 128]
                            ll = wl_sb[:, dc, hc * 128:hc * 128 + 128]
                            nc.tensor.matmul(ps, lw, ith_sb[:, dc, t_sl], start=first, stop=False)
                            nc.tensor.matmul(ps, lw, itl_sb[:, dc, t_sl], start=False, stop=False)
                            nc.tensor.matmul(ps, ll, ith_sb[:, dc, t_sl], start=False, stop=last)
                        nc.vector.tensor_copy(kTh_sb[:, hc, t_sl], ps)
                        nc.vector.tensor_sub(kTl_sb[:, hc, t_sl], ps, kTh_sb[:, hc, t_sl])

            # ================= Phase B: attention + MLP =================
            with tc.tile_pool(name="pb_big", bufs=1) as pb_big, \
                 tc.tile_pool(name="pb_str", bufs=2) as pb_str, \
                 tc.tile_pool(name="pb_att", bufs=2) as pb_att, \
                 tc.tile_pool(name="pb_one", bufs=1) as pb_one, \
                 tc.tile_pool(name="pb_st", bufs=2) as pb_st, \
                 tc.tile_pool(name="pb_sc", bufs=1, space="PSUM") as pb_sc, \
                 tc.tile_pool(name="pb_mm", bufs=2, space="PSUM") as pb_mm:
                vls_sb = pb_big.tile([128, ST, D], BF16)    # 32 KB/part
                mlp_sb = pb_big.tile([128, DC, D], BF16)    # 16 KB/part
                nc.sync.dma_start(out=vls_sb, in_=vls_d)
                nc.sync.dma_start(out=mlp_sb, in_=pcv(mlpb, D))

                for g in range(4):        # s-groups of 512
                    gs = slice(g * 512, g * 512 + 512)
                    attT_t = pb_one.tile([128, ST, 512], BF16, name="attT", tag="attT")   # 16 KB
                    itg_t = pb_one.tile([128, DC, 512], BF16, name="itg", tag="itg")      # 8 KB
                    ret_t = pb_one.tile([128, DC, 512], BF16, name="ret", tag="ret")      # 8 KB
                    nc.sync.dma_start(out=itg_t, in_=iThv[:, :, gs])

                    for st4 in range(4):
                        si = g * 4 + st4
                        s_sl = slice(si * 128, si * 128 + 128)
                        qtsh = pb_str.tile([128, DC, 128], BF16, name="qtsh", tag="qtsh")
                        qtsl = pb_str.tile([128, DC, 128], BF16, name="qtsl", tag="qtsl")
                        nc.sync.dma_start(out=qtsh, in_=qTh_d[:, :, s_sl])
                        nc.sync.dma_start(out=qtsl, in_=qTl_d[:, :, s_sl])

                        scs = [
                            pb_sc.tile([128, 512], F32, name=f"sc{tb}", tag=f"sc{tb}")
                            for tb in range(TB)
                        ]
                        for hc in range(DC):
                            first = hc == 0
                            last = hc == DC - 1
                            for tb in range(TB):
                                t_sl = slice(tb * 512, tb * 512 + 512)
                                nc.tensor.matmul(scs[tb], qtsh[:, hc], kTh_sb[:, hc, t_sl], start=first, stop=False)
                                nc.tensor.matmul(scs[tb], qtsh[:, hc], kTl_sb[:, hc, t_sl], start=False, stop=False)
                                nc.tensor.matmul(scs[tb], qtsl[:, hc], kTh_sb[:, hc, t_sl], start=False, stop=last)

                        st_t = pb_st.tile([128, 12], F32, name="st_t", tag="stats")
                        mx4 = st_t[:, 0:4]
                        sums = st_t[:, 4:8]
                        negmax = st_t[:, 8:9]
                        ssum = st_t[:, 9:10]
                        recip = st_t[:, 10:11]
                        for tb in range(TB):
                            nc.vector.reduce_max(mx4[:, tb:tb + 1], scs[tb], axis=Axis.X)
                        nc.vector.reduce_max(negmax, mx4, axis=Axis.X, negate=True)
                        for tb in range(TB):
                            nc.scalar.activation(
                                out=scs[tb], in_=scs[tb], func=Act.Exp,
                                bias=negmax, scale=1.0,
                                accum_out=sums[:, tb:tb + 1],
                            )
                        nc.vector.reduce_sum(ssum, sums, axis=Axis.X)
                        nc.vector.reciprocal(recip, ssum)

                        att_t = pb_att.tile([128, LN], BF16, name="att_t", tag="att")
                        for tb in range(TB):
                            nc.vector.tensor_scalar_mul(
                                att_t[:, tb * 512:tb * 512 + 512], scs[tb], recip
                            )
                        nc.sync.dma_start_transpose(
                            out=attT_t[:, :, st4 * 128:st4 * 128 + 128], in_=att_t
                        )

                    # att @ vls (+ residual) -> retT[e, s-block]
                    for ec in range(DC):
                        psa = pb_mm.tile([128, 512], F32, name="psa", tag="av")
                        for tc_ in range(ST):
                            nc.tensor.matmul(
                                psa,
                                vls_sb[:, tc_, ec * 128:ec * 128 + 128],
                                attT_t[:, tc_, :],
                                start=(tc_ == 0), stop=(tc_ == ST - 1),
                            )
                        nc.vector.tensor_add(ret_t[:, ec, :], psa, itg_t[:, ec, :])

                    # (ret @ mlp) -> leaky relu -> + bias -> out
                    for st4 in range(4):
                        si = g * 4 + st4
                        s_sl = slice(si * 128, si * 128 + 128)
                        bias_t = pb_str.tile([128, D], F32, name="bias_t", tag="bias")
                        nc.sync.dma_start(out=bias_t, in_=bias.ap()[s_sl, :])
                        out_t = pb_str.tile([128, D], F32, name="out_t", tag="out")
                        for ob in range(2):
                            pso = pb_mm.tile([128, 512], F32, name="pso", tag="om")
                            o_sl = slice(ob * 512, ob * 512 + 512)
                            for ec in range(DC):
                                nc.tensor.matmul(
                                    pso,
                                    ret_t[:, ec, st4 * 128:st4 * 128 + 128],
                                    mlp_sb[:, ec, o_sl],
                                    start=(ec == 0), stop=(ec == DC - 1),
                                )
                            nc.scalar.activation(
                                out=out_t[:, o_sl], in_=pso, func=Act.Prelu,
                                bias=0.0, scale=1.0, alpha=alpha_ap,
                            )
                        nc.vector.tensor_add(out_t, out_t, bias_t)
                        nc.sync.dma_start(out=out_d.ap()[s_sl, :], in_=out_t)

    nc.compile()
    return nc


def _get_nc():
    global _cached_nc
    if _cached_nc is None:
        _cached_nc = _build()
    return _cached_nc


def _split_bf16(x):
    hi = x.astype(ml_dtypes.bfloat16)
    lo = (x - hi.astype(np.float32)).astype(ml_dtypes.bfloat16)
    return hi, lo


def kernel(i, k, q, v, mlp, bias):
    i = np.asarray(i, dtype=np.float32)
    k = np.asarray(k, dtype=np.float32)
    q = np.asarray(q, dtype=np.float32)
    v = np.asarray(v, dtype=np.float32)
    mlp = np.asarray(mlp, dtype=np.float32)
    bias = np.asarray(bias, dtype=np.float32)

    qh, ql = _split_bf16(q)
    kh, kl = _split_bf16(k)
    vh, vl = _split_bf16(v)
    mlpb = mlp.astype(ml_dtypes.bfloat16)

    shared = dict(qh=qh, ql=ql, kh=kh, kl=kl, vh=vh, vl=vl, mlpb=mlpb, bias=bias)
    in_maps = []
    for b in range(N_CORES):
        iT = np.ascontiguousarray(i[b].T)
        iTh, iTl = _split_bf16(iT)
        in_maps.append(dict(iTh=iTh, iTl=iTl, **shared))

    nc = _get_nc()
    res = bass_utils.run_bass_kernel_spmd(nc, in_maps, core_ids=list(range(N_CORES)))
    return np.stack([res.results[b]["out"] for b in range(N_CORES)])


# revision 3
# speedup vs baseline: 1.0247x; 1.0247x over previous
"""Trainium2 Bass kernel for nn_AttentionBlock (B=8, LN=2048, IDM=HDM=ODM=1024).

Sharding: data-parallel over batch, one batch element per NeuronCore (8 cores).

Per-core computation (batch element b):
    queries = i @ q ; keys = i @ k                    [ln, hdm]
    scores  = queries @ keys.T                        [ln, ln]
    att     = softmax(scores, axis=-1)
    vls     = i @ v                                   [ln, idm]
    ret     = att @ vls + i
    out     = leaky_relu(ret @ mlp, 0.2) + bias

Precision strategy: the softmax exponent amplifies matmul operand
rounding, so the Q/K path (q/k projections and scores) uses 3-pass
bf16 split matmuls (hi/lo decomposition, ~fp32 quality). The value/MLP
path tolerates bf16. All accumulation is fp32 in PSUM.

Layout strategy: everything is computed with the contraction dim on
partitions. The host pre-transposes i (iT = i.T) and pre-splits
operands into bf16 hi/lo pairs; on-chip phases:
  A) kT/qT/vls projections (qT, vls staged via DRAM),
  B) per 128-row s-tile: scores -> softmax -> DMA-transpose(att) ->
     att @ vls (+residual) -> @ mlp -> leaky-relu + bias.
"""
import os
import numpy as np
import ml_dtypes

import concourse.bacc as bacc
import concourse.mybir as mybir
import concourse.tile as tile
from concourse import bass_utils

F32 = mybir.dt.float32
BF16 = mybir.dt.bfloat16
Act = mybir.ActivationFunctionType
Axis = mybir.AxisListType

LN = 2048      # sequence length
D = 1024       # idm = hdm = odm
N_CORES = 8
DC = D // 128      # 8 contraction chunks
ST = LN // 128     # 16 s-tiles
TB = LN // 512     # 4 t-blocks (N=512)
NEG_SLOPE = 0.2

_cached_nc = None


def _build():
    nc = bacc.Bacc("TRN2", target_bir_lowering=False, debug=False)

    # Inputs (per core): host provides iT (= i_b.T) and all weights as
    # bf16 hi/lo splits. bias stays fp32.
    iTh = nc.dram_tensor("iTh", [D, LN], BF16, kind="ExternalInput")
    iTl = nc.dram_tensor("iTl", [D, LN], BF16, kind="ExternalInput")
    qh = nc.dram_tensor("qh", [D, D], BF16, kind="ExternalInput")
    ql = nc.dram_tensor("ql", [D, D], BF16, kind="ExternalInput")
    kh = nc.dram_tensor("kh", [D, D], BF16, kind="ExternalInput")
    kl = nc.dram_tensor("kl", [D, D], BF16, kind="ExternalInput")
    vh = nc.dram_tensor("vh", [D, D], BF16, kind="ExternalInput")
    vl = nc.dram_tensor("vl", [D, D], BF16, kind="ExternalInput")
    mlpb = nc.dram_tensor("mlpb", [D, D], BF16, kind="ExternalInput")
    bias = nc.dram_tensor("bias", [LN, D], F32, kind="ExternalInput")
    out_d = nc.dram_tensor("out", [LN, D], F32, kind="ExternalOutput")

    # [D, X] viewed as [128 partitions, DC chunks, X]
    def pcv(t, x):
        return t.ap().rearrange("(c p) x -> p c x", p=128)

    iThv, iTlv = pcv(iTh, LN), pcv(iTl, LN)

    with tile.TileContext(nc) as tc:
        # --- persistent pool (lives through both phases) ---
        with tc.tile_pool(name="pers", bufs=1) as pers, \
             tc.tile_pool(name="dram", bufs=1, space="DRAM") as dram:
            kTh_sb = pers.tile([128, DC, LN], BF16)   # 32 KB/part
            kTl_sb = pers.tile([128, DC, LN], BF16)   # 32 KB/part
            alpha_ap = pers.tile([128, 1], F32)
            nc.vector.memset(alpha_ap, NEG_SLOPE)

            qTh_d = dram.tile([128, DC, LN], BF16)
            qTl_d = dram.tile([128, DC, LN], BF16)
            vls_sb = pers.tile([128, ST, D], BF16)   # 32 KB/part

            # ================= Phase A: projections =================
            with tc.tile_pool(name="pa_it", bufs=1) as pa_it, \
                 tc.tile_pool(name="pa_w", bufs=1) as pa_w, \
                 tc.tile_pool(name="pa_ev", bufs=1) as pa_ev, \
                 tc.tile_pool(name="pa_ps", bufs=4, space="PSUM") as pa_ps:
                ith_sb = pa_it.tile([128, DC, LN], BF16)
                itl_sb = pa_it.tile([128, DC, LN], BF16)

                def load_w(hi_t, lo_t, chunked=False):
                    wh_sb = pa_w.tile([128, DC, D], BF16, name="wh_sb", tag="wh")
                    wl_sb = pa_w.tile([128, DC, D], BF16, name="wl_sb", tag="wl")
                    if chunked:
                        for dc in range(DC):
                            nc.sync.dma_start(out=wh_sb[:, dc], in_=pcv(hi_t, D)[:, dc])
                            nc.sync.dma_start(out=wl_sb[:, dc], in_=pcv(lo_t, D)[:, dc])
                    else:
                        nc.sync.dma_start(out=wh_sb, in_=pcv(hi_t, D))
                        nc.sync.dma_start(out=wl_sb, in_=pcv(lo_t, D))
                    return wh_sb, wl_sb

                # interleave per-dc chunks so dc=0 deps resolve early
                _wq = load_w(qh, ql, chunked=True)
                for dc in range(DC):
                    nc.sync.dma_start(out=ith_sb[:, dc], in_=iThv[:, dc])
                    nc.sync.dma_start(out=itl_sb[:, dc], in_=iTlv[:, dc])

                # --- qT[h, s] (3-pass split) -> DRAM hi/lo ---
                wh_sb, wl_sb = _wq
                for hc in range(DC):
                    evh = pa_ev.tile([128, TB, 512], BF16, name="evh", tag="evh")
                    evl = pa_ev.tile([128, TB, 512], BF16, name="evl", tag="evl")
                    for sb in range(TB):
                        ps = pa_ps.tile([128, 512], F32, name="ps_prep", tag="prep")
                        s_sl = slice(sb * 512, sb * 512 + 512)
                        for dc in range(DC):
                            first = dc == 0
                            last = dc == DC - 1
                            lw = wh_sb[:, dc, hc * 128:hc * 128 + 128]
                            ll = wl_sb[:, dc, hc * 128:hc * 128 + 128]
                            nc.tensor.matmul(ps, lw, ith_sb[:, dc, s_sl], start=first, stop=False)
                            nc.tensor.matmul(ps, lw, itl_sb[:, dc, s_sl], start=False, stop=False)
                            nc.tensor.matmul(ps, ll, ith_sb[:, dc, s_sl], start=False, stop=last)
                        nc.vector.tensor_copy(evh[:, sb], ps)
                        nc.vector.tensor_sub(evl[:, sb], ps, evh[:, sb])
                    nc.sync.dma_start(out=qTh_d[:, hc, :], in_=evh)
                    nc.sync.dma_start(out=qTl_d[:, hc, :], in_=evl)

                # --- vls[t, e] (2-pass: iT full x vh, iTh x vl) -> DRAM ---
                wh_sb, wl_sb = load_w(vh, vl)
                for tc_ in range(ST):
                    t_sl = slice(tc_ * 128, tc_ * 128 + 128)
                    for eb in range(2):
                        ps = pa_ps.tile([128, 512], F32, name="ps_prep2", tag="prep")
                        e_sl = slice(eb * 512, eb * 512 + 512)
                        for dc in range(DC):
                            first = dc == 0
                            last = dc == DC - 1
                            nc.tensor.matmul(ps, ith_sb[:, dc, t_sl], wh_sb[:, dc, e_sl], start=first, stop=False)
                            nc.tensor.matmul(ps, itl_sb[:, dc, t_sl], wh_sb[:, dc, e_sl], start=False, stop=False)
                            nc.tensor.matmul(ps, ith_sb[:, dc, t_sl], wl_sb[:, dc, e_sl], start=False, stop=last)
                        nc.vector.tensor_copy(vls_sb[:, tc_, e_sl], ps)

                # --- kT[h, t] (3-pass split) -> resident SBUF hi/lo ---
                wh_sb, wl_sb = load_w(kh, kl)
                for hc in range(DC):
                    for tb in range(TB):
                        ps = pa_ps.tile([128, 512], F32, name="ps_prep3", tag="prep")
                        t_sl = slice(tb * 512, tb * 512 + 512)
                        for dc in range(DC):
                            first = dc == 0
                            last = dc == DC - 1
                            lw = wh_sb[:, dc, hc * 128:hc * 128 + 128]
                            ll = wl_sb[:, dc, hc * 128:hc * 128 + 128]
                            nc.tensor.matmul(ps, lw, ith_sb[:, dc, t_sl], start=first, stop=False)
                            nc.tensor.matmul(ps, lw, itl_sb[:, dc, t_sl], start=False, stop=False)
                            nc.tensor.matmul(ps, ll, ith_sb[:, dc, t_sl], start=False, stop=last)
                        nc.vector.tensor_copy(kTh_sb[:, hc, t_sl], ps)
                        nc.vector.tensor_sub(kTl_sb[:, hc, t_sl], ps, kTh_sb[:, hc, t_sl])

            # ================= Phase B: attention + MLP =================
            with tc.tile_pool(name="pb_big", bufs=1) as pb_big, \
                 tc.tile_pool(name="pb_str", bufs=2) as pb_str, \
                 tc.tile_pool(name="pb_att", bufs=2) as pb_att, \
                 tc.tile_pool(name="pb_one", bufs=1) as pb_one, \
                 tc.tile_pool(name="pb_st", bufs=2) as pb_st, \
                 tc.tile_pool(name="pb_sc", bufs=1, space="PSUM") as pb_sc, \
                 tc.tile_pool(name="pb_mm", bufs=2, space="PSUM") as pb_mm:
                mlp_sb = pb_big.tile([128, DC, D], BF16)    # 16 KB/part
                nc.sync.dma_start(out=mlp_sb, in_=pcv(mlpb, D))

                for g in range(4):        # s-groups of 512
                    gs = slice(g * 512, g * 512 + 512)
                    attT_t = pb_one.tile([128, ST, 512], BF16, name="attT", tag="attT")   # 16 KB
                    itg_t = pb_one.tile([128, DC, 512], BF16, name="itg", tag="itg")      # 8 KB
                    ret_t = pb_one.tile([128, DC, 512], BF16, name="ret", tag="ret", bufs=2)  # 8 KB x2
                    nc.sync.dma_start(out=itg_t, in_=iThv[:, :, gs])

                    for st4 in range(4):
                        si = g * 4 + st4
                        s_sl = slice(si * 128, si * 128 + 128)
                        qtsh = pb_str.tile([128, DC, 128], BF16, name="qtsh", tag="qtsh")
                        qtsl = pb_str.tile([128, DC, 128], BF16, name="qtsl", tag="qtsl")
                        nc.sync.dma_start(out=qtsh, in_=qTh_d[:, :, s_sl])
                        nc.sync.dma_start(out=qtsl, in_=qTl_d[:, :, s_sl])

                        scs = [
                            pb_sc.tile([128, 512], F32, name=f"sc{tb}", tag=f"sc{tb}")
                            for tb in range(TB)
                        ]
                        for hc in range(DC):
                            first = hc == 0
                            last = hc == DC - 1
                            for tb in range(TB):
                                t_sl = slice(tb * 512, tb * 512 + 512)
                                nc.tensor.matmul(scs[tb], qtsh[:, hc], kTh_sb[:, hc, t_sl], start=first, stop=False)
                                nc.tensor.matmul(scs[tb], qtsh[:, hc], kTl_sb[:, hc, t_sl], start=False, stop=False)
                                nc.tensor.matmul(scs[tb], qtsl[:, hc], kTh_sb[:, hc, t_sl], start=False, stop=last)

                        st_t = pb_st.tile([128, 12], F32, name="st_t", tag="stats")
                        mx4 = st_t[:, 0:4]
                        sums = st_t[:, 4:8]
                        negmax = st_t[:, 8:9]
                        ssum = st_t[:, 9:10]
                        recip = st_t[:, 10:11]
                        for tb in range(TB):
                            nc.vector.reduce_max(mx4[:, tb:tb + 1], scs[tb], axis=Axis.X)
                        nc.vector.reduce_max(negmax, mx4, axis=Axis.X, negate=True)
                        for tb in range(TB):
                            nc.scalar.activation(
                                out=scs[tb], in_=scs[tb], func=Act.Exp,
                                bias=negmax, scale=1.0,
                                accum_out=sums[:, tb:tb + 1],
                            )
                        nc.vector.reduce_sum(ssum, sums, axis=Axis.X)
                        nc.vector.reciprocal(recip, ssum)

                        att_t = pb_att.tile([128, LN], BF16, name="att_t", tag="att")
                        for tb in range(TB):
                            nc.vector.tensor_scalar_mul(
                                att_t[:, tb * 512:tb * 512 + 512], scs[tb], recip
                            )
                        nc.sync.dma_start_transpose(
                            out=attT_t[:, :, st4 * 128:st4 * 128 + 128], in_=att_t
                        )

                    # att @ vls (+ residual) -> retT[e, s-block]
                    for ec in range(DC):
                        psa = pb_mm.tile([128, 512], F32, name="psa", tag="av")
                        for tc_ in range(ST):
                            nc.tensor.matmul(
                                psa,
                                vls_sb[:, tc_, ec * 128:ec * 128 + 128],
                                attT_t[:, tc_, :],
                                start=(tc_ == 0), stop=(tc_ == ST - 1),
                            )
                        nc.vector.tensor_add(ret_t[:, ec, :], psa, itg_t[:, ec, :])

                    # (ret @ mlp) -> leaky relu -> + bias -> out
                    for st4 in range(4):
                        si = g * 4 + st4
                        s_sl = slice(si * 128, si * 128 + 128)
                        bias_t = pb_str.tile([128, D], F32, name="bias_t", tag="bias")
                        nc.sync.dma_start(out=bias_t, in_=bias.ap()[s_sl, :])
                        out_t = pb_str.tile([128, D], F32, name="out_t", tag="out")
                        for ob in range(2):
                            pso = pb_mm.tile([128, 512], F32, name="pso", tag="om")
                            o_sl = slice(ob * 512, ob * 512 + 512)
                            for ec in range(DC):
                                nc.tensor.matmul(
                                    pso,
                                    ret_t[:, ec, st4 * 128:st4 * 128 + 128],
                                    mlp_sb[:, ec, o_sl],
                                    start=(ec == 0), stop=(ec == DC - 1),
                                )
                            nc.scalar.activation(
                                out=out_t[:, o_sl], in_=pso, func=Act.Prelu,
                                bias=0.0, scale=1.0, alpha=alpha_ap,
                            )
                        nc.vector.tensor_add(out_t, out_t, bias_t)
                        nc.sync.dma_start(out=out_d.ap()[s_sl, :], in_=out_t)

    nc.compile()
    return nc


def _get_nc():
    global _cached_nc
    if _cached_nc is None:
        _cached_nc = _build()
    return _cached_nc


def _split_bf16(x):
    hi = x.astype(ml_dtypes.bfloat16)
    lo = (x - hi.astype(np.float32)).astype(ml_dtypes.bfloat16)
    return hi, lo


def kernel(i, k, q, v, mlp, bias):
    i = np.asarray(i, dtype=np.float32)
    k = np.asarray(k, dtype=np.float32)
    q = np.asarray(q, dtype=np.float32)
    v = np.asarray(v, dtype=np.float32)
    mlp = np.asarray(mlp, dtype=np.float32)
    bias = np.asarray(bias, dtype=np.float32)

    qh, ql = _split_bf16(q)
    kh, kl = _split_bf16(k)
    vh, vl = _split_bf16(v)
    mlpb = mlp.astype(ml_dtypes.bfloat16)

    shared = dict(qh=qh, ql=ql, kh=kh, kl=kl, vh=vh, vl=vl, mlpb=mlpb, bias=bias)
    in_maps = []
    for b in range(N_CORES):
        iT = np.ascontiguousarray(i[b].T)
        iTh, iTl = _split_bf16(iT)
        in_maps.append(dict(iTh=iTh, iTl=iTl, **shared))

    nc = _get_nc()
    res = bass_utils.run_bass_kernel_spmd(nc, in_maps, core_ids=list(range(N_CORES)))
    return np.stack([res.results[b]["out"] for b in range(N_CORES)])


# revision 5
# speedup vs baseline: 1.1878x; 1.1592x over previous
"""Trainium2 Bass kernel for nn_AttentionBlock (B=8, LN=2048, IDM=HDM=ODM=1024).

Sharding: data-parallel over batch, one batch element per NeuronCore (8 cores).

Per-core computation (batch element b):
    queries = i @ q ; keys = i @ k                    [ln, hdm]
    scores  = queries @ keys.T                        [ln, ln]
    att     = softmax(scores, axis=-1)
    vls     = i @ v                                   [ln, idm]
    ret     = att @ vls + i
    out     = leaky_relu(ret @ mlp, 0.2) + bias

Precision strategy: the softmax exponent amplifies matmul operand
rounding, so the Q/K path (q/k projections and scores) uses 3-pass
bf16 split matmuls (hi/lo decomposition, ~fp32 quality). The value/MLP
path tolerates bf16. All accumulation is fp32 in PSUM.

Layout strategy: everything is computed with the contraction dim on
partitions. The host pre-transposes i (iT = i.T) and pre-splits
operands into bf16 hi/lo pairs; on-chip phases:
  A) kT/qT/vls projections (qT, vls staged via DRAM),
  B) per 128-row s-tile: scores -> softmax -> DMA-transpose(att) ->
     att @ vls (+residual) -> @ mlp -> leaky-relu + bias.
"""
import os
import numpy as np
import ml_dtypes

import concourse.bacc as bacc
import concourse.mybir as mybir
import concourse.tile as tile
from concourse import bass_utils

F32 = mybir.dt.float32
BF16 = mybir.dt.bfloat16
Act = mybir.ActivationFunctionType
Axis = mybir.AxisListType

LN = 2048      # sequence length
D = 1024       # idm = hdm = odm
N_CORES = 8
DC = D // 128      # 8 contraction chunks
ST = LN // 128     # 16 s-tiles
TB = LN // 512     # 4 t-blocks (N=512)
NEG_SLOPE = 0.2

_cached_nc = None


def _build():
    nc = bacc.Bacc("TRN2", target_bir_lowering=False, debug=False)

    # Inputs (per core): host provides iT (= i_b.T) and all weights as
    # bf16 hi/lo splits. bias stays fp32.
    iTh = nc.dram_tensor("iTh", [D, LN], BF16, kind="ExternalInput")
    iTl = nc.dram_tensor("iTl", [D, LN], BF16, kind="ExternalInput")
    qh = nc.dram_tensor("qh", [D, D], BF16, kind="ExternalInput")
    ql = nc.dram_tensor("ql", [D, D], BF16, kind="ExternalInput")
    kh = nc.dram_tensor("kh", [D, D], BF16, kind="ExternalInput")
    kl = nc.dram_tensor("kl", [D, D], BF16, kind="ExternalInput")
    vh = nc.dram_tensor("vh", [D, D], BF16, kind="ExternalInput")
    vl = nc.dram_tensor("vl", [D, D], BF16, kind="ExternalInput")
    mlpb = nc.dram_tensor("mlpb", [D, D], BF16, kind="ExternalInput")
    bias = nc.dram_tensor("bias", [LN, D], F32, kind="ExternalInput")
    out_d = nc.dram_tensor("out", [LN, D], F32, kind="ExternalOutput")

    # [D, X] viewed as [128 partitions, DC chunks, X]
    def pcv(t, x):
        return t.ap().rearrange("(c p) x -> p c x", p=128)

    iThv, iTlv = pcv(iTh, LN), pcv(iTl, LN)

    with tile.TileContext(nc) as tc:
        # --- persistent pool (lives through both phases) ---
        with tc.tile_pool(name="pers", bufs=1) as pers, \
             tc.tile_pool(name="dram", bufs=1, space="DRAM") as dram:
            kTh_sb = pers.tile([128, DC, LN], BF16)   # 32 KB/part
            kTl_sb = pers.tile([128, DC, LN], BF16)   # 32 KB/part
            alpha_ap = pers.tile([128, 1], F32)
            nc.vector.memset(alpha_ap, NEG_SLOPE)

            qTh_d = dram.tile([128, DC, LN], BF16)
            qTl_d = dram.tile([128, DC, LN], BF16)
            vls_sb = pers.tile([128, ST, D], BF16)   # 32 KB/part

            # ================= Phase A: projections =================
            with tc.tile_pool(name="pa_it", bufs=1) as pa_it, \
                 tc.tile_pool(name="pa_w", bufs=1) as pa_w, \
                 tc.tile_pool(name="pa_ev", bufs=1) as pa_ev, \
                 tc.tile_pool(name="pa_ps", bufs=4, space="PSUM") as pa_ps:
                ith_sb = pa_it.tile([128, DC, LN], BF16)
                itl_sb = pa_it.tile([128, DC, LN], BF16)

                def load_w(hi_t, lo_t, chunked=False):
                    wh_sb = pa_w.tile([128, DC, D], BF16, name="wh_sb", tag="wh")
                    wl_sb = pa_w.tile([128, DC, D], BF16, name="wl_sb", tag="wl")
                    if chunked:
                        for dc in range(DC):
                            nc.sync.dma_start(out=wh_sb[:, dc], in_=pcv(hi_t, D)[:, dc])
                            nc.sync.dma_start(out=wl_sb[:, dc], in_=pcv(lo_t, D)[:, dc])
                    else:
                        nc.sync.dma_start(out=wh_sb, in_=pcv(hi_t, D))
                        nc.sync.dma_start(out=wl_sb, in_=pcv(lo_t, D))
                    return wh_sb, wl_sb

                # interleave per-dc chunks so dc=0 deps resolve early
                _wq = load_w(qh, ql, chunked=True)
                for dc in range(DC):
                    nc.sync.dma_start(out=ith_sb[:, dc], in_=iThv[:, dc])
                    nc.sync.dma_start(out=itl_sb[:, dc], in_=iTlv[:, dc])

                # --- qT[h, s] (3-pass split) -> DRAM hi/lo ---
                wh_sb, wl_sb = _wq
                for hc in range(DC):
                    evh = pa_ev.tile([128, TB, 512], BF16, name="evh", tag="evh")
                    evl = pa_ev.tile([128, TB, 512], BF16, name="evl", tag="evl")
                    for sb in range(TB):
                        ps = pa_ps.tile([128, 512], F32, name="ps_prep", tag="prep")
                        s_sl = slice(sb * 512, sb * 512 + 512)
                        for dc in range(DC):
                            first = dc == 0
                            last = dc == DC - 1
                            lw = wh_sb[:, dc, hc * 128:hc * 128 + 128]
                            ll = wl_sb[:, dc, hc * 128:hc * 128 + 128]
                            nc.tensor.matmul(ps, lw, ith_sb[:, dc, s_sl], start=first, stop=False)
                            nc.tensor.matmul(ps, lw, itl_sb[:, dc, s_sl], start=False, stop=False)
                            nc.tensor.matmul(ps, ll, ith_sb[:, dc, s_sl], start=False, stop=last)
                        nc.vector.tensor_copy(evh[:, sb], ps)
                        nc.vector.tensor_sub(evl[:, sb], ps, evh[:, sb])
                    nc.sync.dma_start(out=qTh_d[:, hc, :], in_=evh)
                    nc.sync.dma_start(out=qTl_d[:, hc, :], in_=evl)

                # --- kT[h, t] (3-pass split) -> resident SBUF hi/lo ---
                wh_sb, wl_sb = load_w(kh, kl)
                for hc in range(DC):
                    for tb in range(TB):
                        ps = pa_ps.tile([128, 512], F32, name="ps_prep3", tag="prep")
                        t_sl = slice(tb * 512, tb * 512 + 512)
                        for dc in range(DC):
                            first = dc == 0
                            last = dc == DC - 1
                            lw = wh_sb[:, dc, hc * 128:hc * 128 + 128]
                            ll = wl_sb[:, dc, hc * 128:hc * 128 + 128]
                            nc.tensor.matmul(ps, lw, ith_sb[:, dc, t_sl], start=first, stop=False)
                            nc.tensor.matmul(ps, lw, itl_sb[:, dc, t_sl], start=False, stop=False)
                            nc.tensor.matmul(ps, ll, ith_sb[:, dc, t_sl], start=False, stop=last)
                        nc.vector.tensor_copy(kTh_sb[:, hc, t_sl], ps)
                        nc.vector.tensor_sub(kTl_sb[:, hc, t_sl], ps, kTh_sb[:, hc, t_sl])

# --- vls[t, e] (2-pass: iT full x vh, iTh x vl) -> DRAM ---
                wh_sb, wl_sb = load_w(vh, vl)
                for tc_ in range(ST):
                    t_sl = slice(tc_ * 128, tc_ * 128 + 128)
                    for eb in range(2):
                        ps = pa_ps.tile([128, 512], F32, name="ps_prep2", tag="prep")
                        e_sl = slice(eb * 512, eb * 512 + 512)
                        for dc in range(DC):
                            first = dc == 0
                            last = dc == DC - 1
                            nc.tensor.matmul(ps, ith_sb[:, dc, t_sl], wh_sb[:, dc, e_sl], start=first, stop=False)
                            nc.tensor.matmul(ps, ith_sb[:, dc, t_sl], wl_sb[:, dc, e_sl], start=False, stop=last)
                        nc.vector.tensor_copy(vls_sb[:, tc_, e_sl], ps)

                            # ================= Phase B: attention + MLP =================
            with tc.tile_pool(name="pb_big", bufs=1) as pb_big, \
                 tc.tile_pool(name="pb_str", bufs=2) as pb_str, \
                 tc.tile_pool(name="pb_att", bufs=2) as pb_att, \
                 tc.tile_pool(name="pb_one", bufs=1) as pb_one, \
                 tc.tile_pool(name="pb_st", bufs=2) as pb_st, \
                 tc.tile_pool(name="pb_sc", bufs=1, space="PSUM") as pb_sc, \
                 tc.tile_pool(name="pb_mm", bufs=2, space="PSUM") as pb_mm:
                mlp_sb = pb_big.tile([128, DC, D], BF16)    # 16 KB/part
                nc.sync.dma_start(out=mlp_sb, in_=pcv(mlpb, D))

                for g in range(4):        # s-groups of 512
                    gs = slice(g * 512, g * 512 + 512)
                    attT_t = pb_one.tile([128, ST, 512], BF16, name="attT", tag="attT")   # 16 KB
                    itg_t = pb_one.tile([128, DC, 512], BF16, name="itg", tag="itg")      # 8 KB
                    ret_t = pb_one.tile([128, DC, 512], BF16, name="ret", tag="ret", bufs=2)  # 8 KB x2
                    nc.sync.dma_start(out=itg_t, in_=iThv[:, :, gs])

                    for st4 in range(4):
                        si = g * 4 + st4
                        s_sl = slice(si * 128, si * 128 + 128)
                        qtsh = pb_str.tile([128, DC, 128], BF16, name="qtsh", tag="qtsh")
                        qtsl = pb_str.tile([128, DC, 128], BF16, name="qtsl", tag="qtsl")
                        nc.sync.dma_start(out=qtsh, in_=qTh_d[:, :, s_sl])
                        nc.sync.dma_start(out=qtsl, in_=qTl_d[:, :, s_sl])

                        scs = [
                            pb_sc.tile([128, 512], F32, name=f"sc{tb}", tag=f"sc{tb}")
                            for tb in range(TB)
                        ]
                        for hc in range(DC):
                            first = hc == 0
                            last = hc == DC - 1
                            for tb in range(TB):
                                t_sl = slice(tb * 512, tb * 512 + 512)
                                nc.tensor.matmul(scs[tb], qtsh[:, hc], kTh_sb[:, hc, t_sl], start=first, stop=False)
                                nc.tensor.matmul(scs[tb], qtsh[:, hc], kTl_sb[:, hc, t_sl], start=False, stop=False)
                                nc.tensor.matmul(scs[tb], qtsl[:, hc], kTh_sb[:, hc, t_sl], start=False, stop=last)

                        # Per-t-block softmax: local max + exp immediately
                        # (frees each PSUM bank early), then algebraic
                        # rescale by f_tb = e^(m_tb - M) / S.
                        st_t = pb_st.tile([128, 24], F32, name="st_t", tag="stats")
                        negm4 = st_t[:, 0:4]
                        sums = st_t[:, 4:8]
                        negM = st_t[:, 8:9]
                        S = st_t[:, 9:10]
                        recip = st_t[:, 10:11]
                        g4 = st_t[:, 12:16]
                        f4 = st_t[:, 16:20]
                        gs = st_t[:, 20:24]
                        att32 = pb_att.tile([128, LN], F32, name="att32", tag="att32", bufs=1)
                        for tb in range(TB):
                            nc.vector.reduce_max(negm4[:, tb:tb + 1], scs[tb], axis=Axis.X, negate=True)
                            nc.scalar.activation(
                                out=att32[:, tb * 512:tb * 512 + 512], in_=scs[tb],
                                func=Act.Exp, bias=negm4[:, tb:tb + 1], scale=1.0,
                                accum_out=sums[:, tb:tb + 1],
                            )
                        nc.vector.tensor_reduce(negM, negm4, axis=Axis.X, op=mybir.AluOpType.min)
                        nc.scalar.activation(out=g4, in_=negm4, func=Act.Exp, bias=negM, scale=-1.0)
                        nc.vector.tensor_mul(gs, g4, sums)
                        nc.vector.reduce_sum(S, gs, axis=Axis.X)
                        nc.vector.reciprocal(recip, S)
                        nc.vector.tensor_scalar_mul(f4, g4, recip)

                        att_t = pb_att.tile([128, LN], BF16, name="att_t", tag="att")
                        for tb in range(TB):
                            nc.vector.tensor_scalar_mul(
                                att_t[:, tb * 512:tb * 512 + 512],
                                att32[:, tb * 512:tb * 512 + 512],
                                f4[:, tb:tb + 1],
                            )
                        nc.sync.dma_start_transpose(
                            out=attT_t[:, :, st4 * 128:st4 * 128 + 128], in_=att_t
                        )

                    # att @ vls (+ residual) -> retT[e, s-block]
                    for ec in range(DC):
                        psa = pb_mm.tile([128, 512], F32, name="psa", tag="av")
                        for tc_ in range(ST):
                            nc.tensor.matmul(
                                psa,
                                vls_sb[:, tc_, ec * 128:ec * 128 + 128],
                                attT_t[:, tc_, :],
                                start=(tc_ == 0), stop=(tc_ == ST - 1),
                            )
                        nc.vector.tensor_add(ret_t[:, ec, :], psa, itg_t[:, ec, :])

                    # (ret @ mlp) -> leaky relu -> + bias -> out
                    for st4 in range(4):
                        si = g * 4 + st4
                        s_sl = slice(si * 128, si * 128 + 128)
                        bias_t = pb_str.tile([128, D], F32, name="bias_t", tag="bias")
                        nc.sync.dma_start(out=bias_t, in_=bias.ap()[s_sl, :])
                        out_t = pb_str.tile([128, D], F32, name="out_t", tag="out")
                        for ob in range(2):
                            pso = pb_mm.tile([128, 512], F32, name="pso", tag="om")
                            o_sl = slice(ob * 512, ob * 512 + 512)
                            for ec in range(DC):
                                nc.tensor.matmul(
                                    pso,
                                    ret_t[:, ec, st4 * 128:st4 * 128 + 128],
                                    mlp_sb[:, ec, o_sl],
                                    start=(ec == 0), stop=(ec == DC - 1),
                                )
                            nc.scalar.activation(
                                out=out_t[:, o_sl], in_=pso, func=Act.Prelu,
                                bias=0.0, scale=1.0, alpha=alpha_ap,
                            )
                        nc.vector.tensor_add(out_t, out_t, bias_t)
                        nc.sync.dma_start(out=out_d.ap()[s_sl, :], in_=out_t)

    nc.compile()
    return nc


def _get_nc():
    global _cached_nc
    if _cached_nc is None:
        _cached_nc = _build()
    return _cached_nc


def _split_bf16(x):
    hi = x.astype(ml_dtypes.bfloat16)
    lo = (x - hi.astype(np.float32)).astype(ml_dtypes.bfloat16)
    return hi, lo


def kernel(i, k, q, v, mlp, bias):
    i = np.asarray(i, dtype=np.float32)
    k = np.asarray(k, dtype=np.float32)
    q = np.asarray(q, dtype=np.float32)
    v = np.asarray(v, dtype=np.float32)
    mlp = np.asarray(mlp, dtype=np.float32)
    bias = np.asarray(bias, dtype=np.float32)

    qh, ql = _split_bf16(q)
    kh, kl = _split_bf16(k)
    vh, vl = _split_bf16(v)
    mlpb = mlp.astype(ml_dtypes.bfloat16)

    shared = dict(qh=qh, ql=ql, kh=kh, kl=kl, vh=vh, vl=vl, mlpb=mlpb, bias=bias)
    in_maps = []
    for b in range(N_CORES):
        iT = np.ascontiguousarray(i[b].T)
        iTh, iTl = _split_bf16(iT)
        in_maps.append(dict(iTh=iTh, iTl=iTl, **shared))

    nc = _get_nc()
    res = bass_utils.run_bass_kernel_spmd(nc, in_maps, core_ids=list(range(N_CORES)))
    return np.stack([res.results[b]["out"] for b in range(N_CORES)])


# revision 7
# speedup vs baseline: 1.1987x; 1.0092x over previous
"""Trainium2 Bass kernel for nn_AttentionBlock (B=8, LN=2048, IDM=HDM=ODM=1024).

Sharding: data-parallel over batch, one batch element per NeuronCore (8 cores).

Per-core computation (batch element b):
    queries = i @ q ; keys = i @ k                    [ln, hdm]
    scores  = queries @ keys.T                        [ln, ln]
    att     = softmax(scores, axis=-1)
    vls     = i @ v                                   [ln, idm]
    ret     = att @ vls + i
    out     = leaky_relu(ret @ mlp, 0.2) + bias

Precision strategy: the softmax exponent amplifies matmul operand
rounding, so the Q/K path (q/k projections and scores) uses 3-pass
bf16 split matmuls (hi/lo decomposition, ~fp32 quality). The value/MLP
path tolerates bf16. All accumulation is fp32 in PSUM.

Layout strategy: everything is computed with the contraction dim on
partitions. The host pre-transposes i (iT = i.T) and pre-splits
operands into bf16 hi/lo pairs; on-chip phases:
  A) kT/qT/vls projections (qT, vls staged via DRAM),
  B) per 128-row s-tile: scores -> softmax -> DMA-transpose(att) ->
     att @ vls (+residual) -> @ mlp -> leaky-relu + bias.
"""
import os
import numpy as np
import ml_dtypes

import concourse.bacc as bacc
import concourse.mybir as mybir
import concourse.tile as tile
from concourse import bass_utils

F32 = mybir.dt.float32
BF16 = mybir.dt.bfloat16
Act = mybir.ActivationFunctionType
Axis = mybir.AxisListType

LN = 2048      # sequence length
D = 1024       # idm = hdm = odm
N_CORES = 8
DC = D // 128      # 8 contraction chunks
ST = LN // 128     # 16 s-tiles
TB = LN // 512     # 4 t-blocks (N=512)
NEG_SLOPE = 0.2

_cached_nc = None


def _build():
    nc = bacc.Bacc("TRN2", target_bir_lowering=False, debug=False)

    # Inputs (per core): host provides iT (= i_b.T) and all weights as
    # bf16 hi/lo splits. bias stays fp32.
    iTh = nc.dram_tensor("iTh", [D, LN], BF16, kind="ExternalInput")
    iTl = nc.dram_tensor("iTl", [D, LN], BF16, kind="ExternalInput")
    qh = nc.dram_tensor("qh", [D, D], BF16, kind="ExternalInput")
    ql = nc.dram_tensor("ql", [D, D], BF16, kind="ExternalInput")
    kh = nc.dram_tensor("kh", [D, D], BF16, kind="ExternalInput")
    kl = nc.dram_tensor("kl", [D, D], BF16, kind="ExternalInput")
    vh = nc.dram_tensor("vh", [D, D], BF16, kind="ExternalInput")
    vl = nc.dram_tensor("vl", [D, D], BF16, kind="ExternalInput")
    mlpb = nc.dram_tensor("mlpb", [D, D], BF16, kind="ExternalInput")
    bias = nc.dram_tensor("bias", [LN, D], F32, kind="ExternalInput")
    out_d = nc.dram_tensor("out", [LN, D], F32, kind="ExternalOutput")

    # [D, X] viewed as [128 partitions, DC chunks, X]
    def pcv(t, x):
        return t.ap().rearrange("(c p) x -> p c x", p=128)

    iThv, iTlv = pcv(iTh, LN), pcv(iTl, LN)

    with tile.TileContext(nc) as tc:
        # --- persistent pool (lives through both phases) ---
        with tc.tile_pool(name="pers", bufs=1) as pers, \
             tc.tile_pool(name="dram", bufs=1, space="DRAM") as dram:
            kTh_sb = pers.tile([128, DC, LN], BF16)   # 32 KB/part
            kTl_sb = pers.tile([128, DC, LN], BF16)   # 32 KB/part
            alpha_ap = pers.tile([128, 1], F32)
            nc.vector.memset(alpha_ap, NEG_SLOPE)

            qTh_d = dram.tile([128, DC, LN], BF16)
            qTl_d = dram.tile([128, DC, LN], BF16)
            vls_sb = pers.tile([128, ST, D], BF16)   # 32 KB/part

            _psum_cm = tc.tile_pool(name="psum", bufs=1, space="PSUM")
            psum_pool = _psum_cm.__enter__()
            _ps_ctr = [0]

            def prep_psum(name):
                i_ = _ps_ctr[0] % 4
                _ps_ctr[0] += 1
                return psum_pool.tile([128, 512], F32, name=f"{name}{_ps_ctr[0]}", tag=f"sc{i_}")

            # ================= Phase A: projections =================
            with tc.tile_pool(name="pa_it", bufs=1) as pa_it, \
                 tc.tile_pool(name="pa_w", bufs=1) as pa_w, \
                 tc.tile_pool(name="pa_ev", bufs=1) as pa_ev:
                ith_sb = pa_it.tile([128, DC, LN], BF16)
                itl_sb = pa_it.tile([128, DC, LN], BF16)

                def load_w(hi_t, lo_t, chunked=False):
                    wh_sb = pa_w.tile([128, DC, D], BF16, name="wh_sb", tag="wh")
                    wl_sb = pa_w.tile([128, DC, D], BF16, name="wl_sb", tag="wl")
                    if chunked:
                        for dc in range(DC):
                            nc.sync.dma_start(out=wh_sb[:, dc], in_=pcv(hi_t, D)[:, dc])
                            nc.sync.dma_start(out=wl_sb[:, dc], in_=pcv(lo_t, D)[:, dc])
                    else:
                        nc.sync.dma_start(out=wh_sb, in_=pcv(hi_t, D))
                        nc.sync.dma_start(out=wl_sb, in_=pcv(lo_t, D))
                    return wh_sb, wl_sb

                # interleave per-dc chunks so dc=0 deps resolve early
                wq_h = pa_w.tile([128, DC, D], BF16, name="wh_sb", tag="wh")
                wq_l = pa_w.tile([128, DC, D], BF16, name="wl_sb", tag="wl")
                for dc in range(DC):
                    nc.sync.dma_start(out=wq_h[:, dc], in_=pcv(qh, D)[:, dc])
                    nc.sync.dma_start(out=ith_sb[:, dc], in_=iThv[:, dc])
                    nc.sync.dma_start(out=wq_l[:, dc], in_=pcv(ql, D)[:, dc])
                    nc.sync.dma_start(out=itl_sb[:, dc], in_=iTlv[:, dc])
                _wq = (wq_h, wq_l)

                # --- qT[h, s] (3-pass split) -> DRAM hi/lo ---
                wh_sb, wl_sb = _wq
                for hc in range(DC):
                    evh = pa_ev.tile([128, TB, 512], BF16, name="evh", tag="evh")
                    evl = pa_ev.tile([128, TB, 512], BF16, name="evl", tag="evl")
                    for sb in range(TB):
                        ps = prep_psum("ps_prep")
                        s_sl = slice(sb * 512, sb * 512 + 512)
                        for dc in range(DC):
                            first = dc == 0
                            last = dc == DC - 1
                            lw = wh_sb[:, dc, hc * 128:hc * 128 + 128]
                            ll = wl_sb[:, dc, hc * 128:hc * 128 + 128]
                            nc.tensor.matmul(ps, lw, ith_sb[:, dc, s_sl], start=first, stop=False)
                            nc.tensor.matmul(ps, lw, itl_sb[:, dc, s_sl], start=False, stop=False)
                            nc.tensor.matmul(ps, ll, ith_sb[:, dc, s_sl], start=False, stop=last)
                        nc.vector.tensor_copy(evh[:, sb], ps)
                        nc.vector.tensor_sub(evl[:, sb], ps, evh[:, sb])
                    nc.sync.dma_start(out=qTh_d[:, hc, :], in_=evh)
                    nc.sync.dma_start(out=qTl_d[:, hc, :], in_=evl)

                # --- kT[h, t] (3-pass split) -> resident SBUF hi/lo ---
                wh_sb, wl_sb = load_w(kh, kl)
                for hc in range(DC):
                    for tb in range(TB):
                        ps = prep_psum("ps_prep3")
                        t_sl = slice(tb * 512, tb * 512 + 512)
                        for dc in range(DC):
                            first = dc == 0
                            last = dc == DC - 1
                            lw = wh_sb[:, dc, hc * 128:hc * 128 + 128]
                            ll = wl_sb[:, dc, hc * 128:hc * 128 + 128]
                            nc.tensor.matmul(ps, lw, ith_sb[:, dc, t_sl], start=first, stop=False)
                            nc.tensor.matmul(ps, lw, itl_sb[:, dc, t_sl], start=False, stop=False)
                            nc.tensor.matmul(ps, ll, ith_sb[:, dc, t_sl], start=False, stop=last)
                        nc.vector.tensor_copy(kTh_sb[:, hc, t_sl], ps)
                        nc.vector.tensor_sub(kTl_sb[:, hc, t_sl], ps, kTh_sb[:, hc, t_sl])

# --- vls[t, e] (2-pass: iT full x vh, iTh x vl) -> DRAM ---
                wh_sb, wl_sb = load_w(vh, vl)
                for tc_ in range(ST):
                    t_sl = slice(tc_ * 128, tc_ * 128 + 128)
                    for eb in range(2):
                        ps = prep_psum("ps_prep2")
                        e_sl = slice(eb * 512, eb * 512 + 512)
                        for dc in range(DC):
                            first = dc == 0
                            last = dc == DC - 1
                            nc.tensor.matmul(ps, ith_sb[:, dc, t_sl], wh_sb[:, dc, e_sl], start=first, stop=False)
                            nc.tensor.matmul(ps, ith_sb[:, dc, t_sl], wl_sb[:, dc, e_sl], start=False, stop=last)
                        nc.vector.tensor_copy(vls_sb[:, tc_, e_sl], ps)

                            # ================= Phase B: attention + MLP =================
            with tc.tile_pool(name="pb_big", bufs=1) as pb_big, \
                 tc.tile_pool(name="pb_str", bufs=2) as pb_str, \
                 tc.tile_pool(name="pb_att", bufs=2) as pb_att, \
                 tc.tile_pool(name="pb_one", bufs=1) as pb_one, \
                 tc.tile_pool(name="pb_st", bufs=2) as pb_st:
                pb_sc = psum_pool
                pb_mm = psum_pool
                mlp_sb = pb_big.tile([128, DC, D], BF16)    # 16 KB/part
                nc.sync.dma_start(out=mlp_sb, in_=pcv(mlpb, D))

                for g in range(4):        # s-groups of 512
                    gs = slice(g * 512, g * 512 + 512)
                    attT_t = pb_one.tile([128, ST, 512], BF16, name="attT", tag="attT")   # 16 KB
                    itg_t = pb_one.tile([128, DC, 512], BF16, name="itg", tag="itg")      # 8 KB
                    ret_t = pb_one.tile([128, DC, 512], BF16, name="ret", tag="ret", bufs=2)  # 8 KB x2
                    nc.sync.dma_start(out=itg_t, in_=iThv[:, :, gs])

                    for st4 in range(4):
                        si = g * 4 + st4
                        s_sl = slice(si * 128, si * 128 + 128)
                        qtsh = pb_str.tile([128, DC, 128], BF16, name="qtsh", tag="qtsh")
                        qtsl = pb_str.tile([128, DC, 128], BF16, name="qtsl", tag="qtsl")
                        nc.sync.dma_start(out=qtsh, in_=qTh_d[:, :, s_sl])
                        nc.sync.dma_start(out=qtsl, in_=qTl_d[:, :, s_sl])

                        scs = [
                            pb_sc.tile([128, 512], F32, name=f"sc{tb}", tag=f"sc{tb}")
                            for tb in range(TB)
                        ]
                        for hc in range(DC):
                            first = hc == 0
                            last = hc == DC - 1
                            for tb in range(TB):
                                t_sl = slice(tb * 512, tb * 512 + 512)
                                nc.tensor.matmul(scs[tb], qtsh[:, hc], kTh_sb[:, hc, t_sl], start=first, stop=False)
                                nc.tensor.matmul(scs[tb], qtsh[:, hc], kTl_sb[:, hc, t_sl], start=False, stop=False)
                                nc.tensor.matmul(scs[tb], qtsl[:, hc], kTh_sb[:, hc, t_sl], start=False, stop=last)

                        # Per-t-block softmax: local max + exp immediately
                        # (frees each PSUM bank early), then algebraic
                        # rescale by f_tb = e^(m_tb - M) / S.
                        st_t = pb_st.tile([128, 24], F32, name="st_t", tag="stats")
                        negm4 = st_t[:, 0:4]
                        sums = st_t[:, 4:8]
                        negM = st_t[:, 8:9]
                        S = st_t[:, 9:10]
                        recip = st_t[:, 10:11]
                        g4 = st_t[:, 12:16]
                        f4 = st_t[:, 16:20]
                        gs = st_t[:, 20:24]
                        att32 = pb_att.tile([128, LN], F32, name="att32", tag="att32", bufs=1)
                        for tb in range(TB):
                            nc.vector.reduce_max(negm4[:, tb:tb + 1], scs[tb], axis=Axis.X, negate=True)
                            nc.scalar.activation(
                                out=att32[:, tb * 512:tb * 512 + 512], in_=scs[tb],
                                func=Act.Exp, bias=negm4[:, tb:tb + 1], scale=1.0,
                                accum_out=sums[:, tb:tb + 1],
                            )
                        nc.vector.tensor_reduce(negM, negm4, axis=Axis.X, op=mybir.AluOpType.min)
                        nc.scalar.activation(out=g4, in_=negm4, func=Act.Exp, bias=negM, scale=-1.0)
                        nc.vector.tensor_mul(gs, g4, sums)
                        nc.vector.reduce_sum(S, gs, axis=Axis.X)
                        nc.vector.reciprocal(recip, S)
                        nc.vector.tensor_scalar_mul(f4, g4, recip)

                        att_t = pb_att.tile([128, LN], BF16, name="att_t", tag="att")
                        for tb in range(TB):
                            nc.vector.tensor_scalar_mul(
                                att_t[:, tb * 512:tb * 512 + 512],
                                att32[:, tb * 512:tb * 512 + 512],
                                f4[:, tb:tb + 1],
                            )
                        nc.sync.dma_start_transpose(
                            out=attT_t[:, :, st4 * 128:st4 * 128 + 128], in_=att_t
                        )

                    # att @ vls (+ residual) -> retT[e, s-block]
                    for ec in range(DC):
                        psa = pb_mm.tile([128, 512], F32, name="psa", tag="av", bufs=2)
                        for tc_ in range(ST):
                            nc.tensor.matmul(
                                psa,
                                vls_sb[:, tc_, ec * 128:ec * 128 + 128],
                                attT_t[:, tc_, :],
                                start=(tc_ == 0), stop=(tc_ == ST - 1),
                            )
                        nc.vector.tensor_add(ret_t[:, ec, :], psa, itg_t[:, ec, :])

                    # (ret @ mlp) -> leaky relu -> + bias -> out
                    for st4 in range(4):
                        si = g * 4 + st4
                        s_sl = slice(si * 128, si * 128 + 128)
                        bias_t = pb_str.tile([128, D], F32, name="bias_t", tag="bias")
                        nc.sync.dma_start(out=bias_t, in_=bias.ap()[s_sl, :])
                        out_t = pb_str.tile([128, D], F32, name="out_t", tag="out")
                        for ob in range(2):
                            pso = pb_mm.tile([128, 512], F32, name="pso", tag="om", bufs=2)
                            o_sl = slice(ob * 512, ob * 512 + 512)
                            for ec in range(DC):
                                nc.tensor.matmul(
                                    pso,
                                    ret_t[:, ec, st4 * 128:st4 * 128 + 128],
                                    mlp_sb[:, ec, o_sl],
                                    start=(ec == 0), stop=(ec == DC - 1),
                                )
                            nc.scalar.activation(
                                out=out_t[:, o_sl], in_=pso, func=Act.Prelu,
                                bias=0.0, scale=1.0, alpha=alpha_ap,
                            )
                        nc.vector.tensor_add(out_t, out_t, bias_t)
                        nc.sync.dma_start(out=out_d.ap()[s_sl, :], in_=out_t)

            _psum_cm.__exit__(None, None, None)

    nc.compile()
    return nc


def _get_nc():
    global _cached_nc
    if _cached_nc is None:
        _cached_nc = _build()
    return _cached_nc


def _split_bf16(x):
    hi = x.astype(ml_dtypes.bfloat16)
    lo = (x - hi.astype(np.float32)).astype(ml_dtypes.bfloat16)
    return hi, lo


def kernel(i, k, q, v, mlp, bias):
    i = np.asarray(i, dtype=np.float32)
    k = np.asarray(k, dtype=np.float32)
    q = np.asarray(q, dtype=np.float32)
    v = np.asarray(v, dtype=np.float32)
    mlp = np.asarray(mlp, dtype=np.float32)
    bias = np.asarray(bias, dtype=np.float32)

    qh, ql = _split_bf16(q)
    kh, kl = _split_bf16(k)
    vh, vl = _split_bf16(v)
    mlpb = mlp.astype(ml_dtypes.bfloat16)

    shared = dict(qh=qh, ql=ql, kh=kh, kl=kl, vh=vh, vl=vl, mlpb=mlpb, bias=bias)
    in_maps = []
    for b in range(N_CORES):
        iT = np.ascontiguousarray(i[b].T)
        iTh, iTl = _split_bf16(iT)
        in_maps.append(dict(iTh=iTh, iTl=iTl, **shared))

    nc = _get_nc()
    res = bass_utils.run_bass_kernel_spmd(nc, in_maps, core_ids=list(range(N_CORES)))
    return np.stack([res.results[b]["out"] for b in range(N_CORES)])


# revision 8
# speedup vs baseline: 1.1993x; 1.0005x over previous
"""Trainium2 Bass kernel for nn_AttentionBlock (B=8, LN=2048, IDM=HDM=ODM=1024).

Sharding: data-parallel over batch, one batch element per NeuronCore (8 cores).

Per-core computation (batch element b):
    queries = i @ q ; keys = i @ k                    [ln, hdm]
    scores  = queries @ keys.T                        [ln, ln]
    att     = softmax(scores, axis=-1)
    vls     = i @ v                                   [ln, idm]
    ret     = att @ vls + i
    out     = leaky_relu(ret @ mlp, 0.2) + bias

Precision strategy: the softmax exponent amplifies matmul operand
rounding, so the Q/K path (q/k projections and scores) uses 3-pass
bf16 split matmuls (hi/lo decomposition, ~fp32 quality). The value/MLP
path tolerates bf16. All accumulation is fp32 in PSUM.

Layout strategy: everything is computed with the contraction dim on
partitions. The host pre-transposes i (iT = i.T) and pre-splits
operands into bf16 hi/lo pairs; on-chip phases:
  A) kT/qT/vls projections (qT, vls staged via DRAM),
  B) per 128-row s-tile: scores -> softmax -> DMA-transpose(att) ->
     att @ vls (+residual) -> @ mlp -> leaky-relu + bias.
"""
import os
import numpy as np
import ml_dtypes

import concourse.bacc as bacc
import concourse.mybir as mybir
import concourse.tile as tile
from concourse import bass_utils

F32 = mybir.dt.float32
BF16 = mybir.dt.bfloat16
Act = mybir.ActivationFunctionType
Axis = mybir.AxisListType

LN = 2048      # sequence length
D = 1024       # idm = hdm = odm
N_CORES = 8
DC = D // 128      # 8 contraction chunks
ST = LN // 128     # 16 s-tiles
TB = LN // 512     # 4 t-blocks (N=512)
NEG_SLOPE = 0.2

_cached_nc = None


def _build():
    nc = bacc.Bacc("TRN2", target_bir_lowering=False, debug=False)

    # Inputs (per core): host provides iT (= i_b.T) and all weights as
    # bf16 hi/lo splits. bias stays fp32.
    iTh = nc.dram_tensor("iTh", [D, LN], BF16, kind="ExternalInput")
    iTl = nc.dram_tensor("iTl", [D, LN], BF16, kind="ExternalInput")
    qh = nc.dram_tensor("qh", [D, D], BF16, kind="ExternalInput")
    ql = nc.dram_tensor("ql", [D, D], BF16, kind="ExternalInput")
    kh = nc.dram_tensor("kh", [D, D], BF16, kind="ExternalInput")
    kl = nc.dram_tensor("kl", [D, D], BF16, kind="ExternalInput")
    vh = nc.dram_tensor("vh", [D, D], BF16, kind="ExternalInput")
    vl = nc.dram_tensor("vl", [D, D], BF16, kind="ExternalInput")
    mlpb = nc.dram_tensor("mlpb", [D, D], BF16, kind="ExternalInput")
    bias = nc.dram_tensor("bias", [LN, D], F32, kind="ExternalInput")
    out_d = nc.dram_tensor("out", [LN, D], F32, kind="ExternalOutput")

    # [D, X] viewed as [128 partitions, DC chunks, X]
    def pcv(t, x):
        return t.ap().rearrange("(c p) x -> p c x", p=128)

    iThv, iTlv = pcv(iTh, LN), pcv(iTl, LN)

    with tile.TileContext(nc) as tc:
        # --- persistent pool (lives through both phases) ---
        with tc.tile_pool(name="pers", bufs=1) as pers, \
             tc.tile_pool(name="dram", bufs=1, space="DRAM") as dram:
            kTh_sb = pers.tile([128, DC, LN], BF16)   # 32 KB/part
            kTl_sb = pers.tile([128, DC, LN], BF16)   # 32 KB/part
            alpha_ap = pers.tile([128, 1], F32)
            nc.vector.memset(alpha_ap, NEG_SLOPE)

            qTh_d = dram.tile([ST, 128, DC, 128], BF16)
            qTl_d = dram.tile([ST, 128, DC, 128], BF16)
            vls_sb = pers.tile([128, ST, D], BF16)   # 32 KB/part

            _psum_cm = tc.tile_pool(name="psum", bufs=1, space="PSUM")
            psum_pool = _psum_cm.__enter__()
            _ps_ctr = [0]

            def prep_psum(name):
                i_ = _ps_ctr[0] % 4
                _ps_ctr[0] += 1
                return psum_pool.tile([128, 512], F32, name=f"{name}{_ps_ctr[0]}", tag=f"sc{i_}")

            # ================= Phase A: projections =================
            with tc.tile_pool(name="pa_it", bufs=1) as pa_it, \
                 tc.tile_pool(name="pa_w", bufs=1) as pa_w, \
                 tc.tile_pool(name="pa_ev", bufs=1) as pa_ev:
                ith_sb = pa_it.tile([128, DC, LN], BF16)
                itl_sb = pa_it.tile([128, DC, LN], BF16)

                def load_w(hi_t, lo_t, chunked=False):
                    wh_sb = pa_w.tile([128, DC, D], BF16, name="wh_sb", tag="wh")
                    wl_sb = pa_w.tile([128, DC, D], BF16, name="wl_sb", tag="wl")
                    if chunked:
                        for dc in range(DC):
                            nc.sync.dma_start(out=wh_sb[:, dc], in_=pcv(hi_t, D)[:, dc])
                            nc.sync.dma_start(out=wl_sb[:, dc], in_=pcv(lo_t, D)[:, dc])
                    else:
                        nc.sync.dma_start(out=wh_sb, in_=pcv(hi_t, D))
                        nc.sync.dma_start(out=wl_sb, in_=pcv(lo_t, D))
                    return wh_sb, wl_sb

                # interleave per-dc chunks so dc=0 deps resolve early
                wq_h = pa_w.tile([128, DC, D], BF16, name="wh_sb", tag="wh")
                wq_l = pa_w.tile([128, DC, D], BF16, name="wl_sb", tag="wl")
                for dc in range(DC):
                    nc.sync.dma_start(out=wq_h[:, dc], in_=pcv(qh, D)[:, dc])
                    nc.sync.dma_start(out=ith_sb[:, dc], in_=iThv[:, dc])
                    nc.sync.dma_start(out=wq_l[:, dc], in_=pcv(ql, D)[:, dc])
                    nc.sync.dma_start(out=itl_sb[:, dc], in_=iTlv[:, dc])
                _wq = (wq_h, wq_l)

                # --- qT[h, s] (3-pass split) -> DRAM hi/lo ---
                wh_sb, wl_sb = _wq
                for hc in range(DC):
                    evh = pa_ev.tile([128, TB, 512], BF16, name="evh", tag="evh")
                    evl = pa_ev.tile([128, TB, 512], BF16, name="evl", tag="evl")
                    for sb in range(TB):
                        ps = prep_psum("ps_prep")
                        s_sl = slice(sb * 512, sb * 512 + 512)
                        for dc in range(DC):
                            first = dc == 0
                            last = dc == DC - 1
                            lw = wh_sb[:, dc, hc * 128:hc * 128 + 128]
                            ll = wl_sb[:, dc, hc * 128:hc * 128 + 128]
                            nc.tensor.matmul(ps, lw, ith_sb[:, dc, s_sl], start=first, stop=False)
                            nc.tensor.matmul(ps, lw, itl_sb[:, dc, s_sl], start=False, stop=False)
                            nc.tensor.matmul(ps, ll, ith_sb[:, dc, s_sl], start=False, stop=last)
                        nc.vector.tensor_copy(evh[:, sb], ps)
                        nc.vector.tensor_sub(evl[:, sb], ps, evh[:, sb])
                    evh_v = evh.rearrange("p b (si x) -> p (b si) x", x=128)
                    evl_v = evl.rearrange("p b (si x) -> p (b si) x", x=128)
                    for si16 in range(ST):
                        nc.sync.dma_start(out=qTh_d[si16, :, hc, :], in_=evh_v[:, si16])
                        nc.sync.dma_start(out=qTl_d[si16, :, hc, :], in_=evl_v[:, si16])

                # --- kT[h, t] (3-pass split) -> resident SBUF hi/lo ---
                wh_sb, wl_sb = load_w(kh, kl, chunked=True)
                for hc in range(DC):
                    for tb in range(TB):
                        ps = prep_psum("ps_prep3")
                        t_sl = slice(tb * 512, tb * 512 + 512)
                        for dc in range(DC):
                            first = dc == 0
                            last = dc == DC - 1
                            lw = wh_sb[:, dc, hc * 128:hc * 128 + 128]
                            ll = wl_sb[:, dc, hc * 128:hc * 128 + 128]
                            nc.tensor.matmul(ps, lw, ith_sb[:, dc, t_sl], start=first, stop=False)
                            nc.tensor.matmul(ps, lw, itl_sb[:, dc, t_sl], start=False, stop=False)
                            nc.tensor.matmul(ps, ll, ith_sb[:, dc, t_sl], start=False, stop=last)
                        nc.vector.tensor_copy(kTh_sb[:, hc, t_sl], ps)
                        nc.vector.tensor_sub(kTl_sb[:, hc, t_sl], ps, kTh_sb[:, hc, t_sl])

# --- vls[t, e] (2-pass: iT full x vh, iTh x vl) -> DRAM ---
                wh_sb, wl_sb = load_w(vh, vl, chunked=True)
                for tc_ in range(ST):
                    t_sl = slice(tc_ * 128, tc_ * 128 + 128)
                    for eb in range(2):
                        ps = prep_psum("ps_prep2")
                        e_sl = slice(eb * 512, eb * 512 + 512)
                        for dc in range(DC):
                            first = dc == 0
                            last = dc == DC - 1
                            nc.tensor.matmul(ps, ith_sb[:, dc, t_sl], wh_sb[:, dc, e_sl], start=first, stop=False)
                            nc.tensor.matmul(ps, ith_sb[:, dc, t_sl], wl_sb[:, dc, e_sl], start=False, stop=last)
                        nc.vector.tensor_copy(vls_sb[:, tc_, e_sl], ps)

                            # ================= Phase B: attention + MLP =================
            with tc.tile_pool(name="pb_big", bufs=1) as pb_big, \
                 tc.tile_pool(name="pb_str", bufs=2) as pb_str, \
                 tc.tile_pool(name="pb_att", bufs=2) as pb_att, \
                 tc.tile_pool(name="pb_one", bufs=1) as pb_one, \
                 tc.tile_pool(name="pb_st", bufs=2) as pb_st:
                pb_sc = psum_pool
                pb_mm = psum_pool
                mlp_sb = pb_big.tile([128, DC, D], BF16)    # 16 KB/part
                nc.sync.dma_start(out=mlp_sb, in_=pcv(mlpb, D))

                for g in range(4):        # s-groups of 512
                    gs = slice(g * 512, g * 512 + 512)
                    attT_t = pb_one.tile([128, ST, 512], BF16, name="attT", tag="attT")   # 16 KB
                    itg_t = pb_one.tile([128, DC, 512], BF16, name="itg", tag="itg")      # 8 KB
                    ret_t = pb_one.tile([128, DC, 512], BF16, name="ret", tag="ret", bufs=2)  # 8 KB x2
                    nc.sync.dma_start(out=itg_t, in_=iThv[:, :, gs])

                    for st4 in range(4):
                        si = g * 4 + st4
                        s_sl = slice(si * 128, si * 128 + 128)
                        qtsh = pb_str.tile([128, DC, 128], BF16, name="qtsh", tag="qtsh")
                        qtsl = pb_str.tile([128, DC, 128], BF16, name="qtsl", tag="qtsl")
                        nc.sync.dma_start(out=qtsh, in_=qTh_d[si])
                        nc.sync.dma_start(out=qtsl, in_=qTl_d[si])

                        scs = [
                            pb_sc.tile([128, 512], F32, name=f"sc{tb}", tag=f"sc{tb}")
                            for tb in range(TB)
                        ]
                        for hc in range(DC):
                            first = hc == 0
                            last = hc == DC - 1
                            for tb in range(TB):
                                t_sl = slice(tb * 512, tb * 512 + 512)
                                nc.tensor.matmul(scs[tb], qtsh[:, hc], kTh_sb[:, hc, t_sl], start=first, stop=False)
                                nc.tensor.matmul(scs[tb], qtsh[:, hc], kTl_sb[:, hc, t_sl], start=False, stop=False)
                                nc.tensor.matmul(scs[tb], qtsl[:, hc], kTh_sb[:, hc, t_sl], start=False, stop=last)

                        # Per-t-block softmax: local max + exp immediately
                        # (frees each PSUM bank early), then algebraic
                        # rescale by f_tb = e^(m_tb - M) / S.
                        st_t = pb_st.tile([128, 24], F32, name="st_t", tag="stats")
                        negm4 = st_t[:, 0:4]
                        sums = st_t[:, 4:8]
                        negM = st_t[:, 8:9]
                        S = st_t[:, 9:10]
                        recip = st_t[:, 10:11]
                        g4 = st_t[:, 12:16]
                        f4 = st_t[:, 16:20]
                        gs = st_t[:, 20:24]
                        att32 = pb_att.tile([128, LN], F32, name="att32", tag="att32", bufs=1)
                        for tb in range(TB):
                            nc.vector.reduce_max(negm4[:, tb:tb + 1], scs[tb], axis=Axis.X, negate=True)
                            nc.scalar.activation(
                                out=att32[:, tb * 512:tb * 512 + 512], in_=scs[tb],
                                func=Act.Exp, bias=negm4[:, tb:tb + 1], scale=1.0,
                                accum_out=sums[:, tb:tb + 1],
                            )
                        nc.vector.tensor_reduce(negM, negm4, axis=Axis.X, op=mybir.AluOpType.min)
                        nc.scalar.activation(out=g4, in_=negm4, func=Act.Exp, bias=negM, scale=-1.0)
                        nc.vector.tensor_mul(gs, g4, sums)
                        nc.vector.reduce_sum(S, gs, axis=Axis.X)
                        nc.vector.reciprocal(recip, S)
                        nc.vector.tensor_scalar_mul(f4, g4, recip)

                        att_t = pb_att.tile([128, LN], BF16, name="att_t", tag="att")
                        for tb in range(TB):
                            nc.vector.tensor_scalar_mul(
                                att_t[:, tb * 512:tb * 512 + 512],
                                att32[:, tb * 512:tb * 512 + 512],
                                f4[:, tb:tb + 1],
                            )
                        nc.sync.dma_start_transpose(
                            out=attT_t[:, :, st4 * 128:st4 * 128 + 128], in_=att_t
                        )

                    # att @ vls (+ residual) -> retT[e, s-block]
                    for ec in range(DC):
                        psa = pb_mm.tile([128, 512], F32, name="psa", tag="av", bufs=2)
                        for tc_ in range(ST):
                            nc.tensor.matmul(
                                psa,
                                vls_sb[:, tc_, ec * 128:ec * 128 + 128],
                                attT_t[:, tc_, :],
                                start=(tc_ == 0), stop=(tc_ == ST - 1),
                            )
                        nc.vector.tensor_add(ret_t[:, ec, :], psa, itg_t[:, ec, :])

                    # (ret @ mlp) -> leaky relu -> + bias -> out
                    for st4 in range(4):
                        si = g * 4 + st4
                        s_sl = slice(si * 128, si * 128 + 128)
                        bias_t = pb_str.tile([128, D], F32, name="bias_t", tag="bias")
                        nc.sync.dma_start(out=bias_t, in_=bias.ap()[s_sl, :])
                        out_t = pb_str.tile([128, D], F32, name="out_t", tag="out")
                        for ob in range(2):
                            pso = pb_mm.tile([128, 512], F32, name="pso", tag="om", bufs=2)
                            o_sl = slice(ob * 512, ob * 512 + 512)
                            for ec in range(DC):
                                nc.tensor.matmul(
                                    pso,
                                    ret_t[:, ec, st4 * 128:st4 * 128 + 128],
                                    mlp_sb[:, ec, o_sl],
                                    start=(ec == 0), stop=(ec == DC - 1),
                                )
                            nc.scalar.activation(
                                out=out_t[:, o_sl], in_=pso, func=Act.Prelu,
                                bias=0.0, scale=1.0, alpha=alpha_ap,
                            )
                        nc.vector.tensor_add(out_t, out_t, bias_t)
                        nc.sync.dma_start(out=out_d.ap()[s_sl, :], in_=out_t)

            _psum_cm.__exit__(None, None, None)

    nc.compile()
    return nc


def _get_nc():
    global _cached_nc
    if _cached_nc is None:
        _cached_nc = _build()
    return _cached_nc


def _split_bf16(x):
    hi = x.astype(ml_dtypes.bfloat16)
    lo = (x - hi.astype(np.float32)).astype(ml_dtypes.bfloat16)
    return hi, lo


def kernel(i, k, q, v, mlp, bias):
    i = np.asarray(i, dtype=np.float32)
    k = np.asarray(k, dtype=np.float32)
    q = np.asarray(q, dtype=np.float32)
    v = np.asarray(v, dtype=np.float32)
    mlp = np.asarray(mlp, dtype=np.float32)
    bias = np.asarray(bias, dtype=np.float32)

    qh, ql = _split_bf16(q)
    kh, kl = _split_bf16(k)
    vh, vl = _split_bf16(v)
    mlpb = mlp.astype(ml_dtypes.bfloat16)

    shared = dict(qh=qh, ql=ql, kh=kh, kl=kl, vh=vh, vl=vl, mlpb=mlpb, bias=bias)
    in_maps = []
    for b in range(N_CORES):
        iT = np.ascontiguousarray(i[b].T)
        iTh, iTl = _split_bf16(iT)
        in_maps.append(dict(iTh=iTh, iTl=iTl, **shared))

    nc = _get_nc()
    res = bass_utils.run_bass_kernel_spmd(nc, in_maps, core_ids=list(range(N_CORES)))
    return np.stack([res.results[b]["out"] for b in range(N_CORES)])


# revision 9
# speedup vs baseline: 1.4462x; 1.2059x over previous
"""Trainium2 Bass kernel for nn_AttentionBlock (B=8, LN=2048, IDM=HDM=ODM=1024).

Sharding: data-parallel over batch, one batch element per NeuronCore (8 cores).

Per-core computation (batch element b):
    queries = i @ q ; keys = i @ k                    [ln, hdm]
    scores  = queries @ keys.T                        [ln, ln]
    att     = softmax(scores, axis=-1)
    vls     = i @ v                                   [ln, idm]
    ret     = att @ vls + i
    out     = leaky_relu(ret @ mlp, 0.2) + bias

Precision strategy: the softmax exponent amplifies matmul operand
rounding, so the Q/K path (q/k projections and scores) uses 3-pass
bf16 split matmuls (hi/lo decomposition, ~fp32 quality). The value/MLP
path tolerates bf16. All accumulation is fp32 in PSUM.

Layout strategy: everything is computed with the contraction dim on
partitions. The host pre-transposes i (iT = i.T) and pre-splits
operands into bf16 hi/lo pairs; on-chip phases:
  A) kT/qT/vls projections (qT, vls staged via DRAM),
  B) per 128-row s-tile: scores -> softmax -> DMA-transpose(att) ->
     att @ vls (+residual) -> @ mlp -> leaky-relu + bias.
"""
import os
import numpy as np
import ml_dtypes

import concourse.bacc as bacc
import concourse.mybir as mybir
import concourse.tile as tile
from concourse import bass_utils

F32 = mybir.dt.float32
BF16 = mybir.dt.bfloat16
Act = mybir.ActivationFunctionType
Axis = mybir.AxisListType

LN = 2048      # sequence length
D = 1024       # idm = hdm = odm
N_CORES = 8
DC = D // 128      # 8 contraction chunks
ST = LN // 128     # 16 s-tiles
TB = LN // 512     # 4 t-blocks (N=512)
NEG_SLOPE = 0.2

_cached_nc = None


def _build():
    nc = bacc.Bacc("TRN2", target_bir_lowering=False, debug=False)

    # Inputs (per core): host provides iT (= i_b.T) and all weights as
    # bf16 hi/lo splits. bias stays fp32.
    iTh = nc.dram_tensor("iTh", [D, LN], BF16, kind="ExternalInput")
    iTl = nc.dram_tensor("iTl", [D, LN], BF16, kind="ExternalInput")
    qh = nc.dram_tensor("qh", [D, D], BF16, kind="ExternalInput")
    ql = nc.dram_tensor("ql", [D, D], BF16, kind="ExternalInput")
    kh = nc.dram_tensor("kh", [D, D], BF16, kind="ExternalInput")
    kl = nc.dram_tensor("kl", [D, D], BF16, kind="ExternalInput")
    vh = nc.dram_tensor("vh", [D, D], BF16, kind="ExternalInput")
    vl = nc.dram_tensor("vl", [D, D], BF16, kind="ExternalInput")
    mlpb = nc.dram_tensor("mlpb", [D, D], BF16, kind="ExternalInput")
    bias = nc.dram_tensor("bias", [LN, D], F32, kind="ExternalInput")
    out_d = nc.dram_tensor("out", [LN, D], F32, kind="ExternalOutput")

    # [D, X] viewed as [128 partitions, DC chunks, X]
    def pcv(t, x):
        return t.ap().rearrange("(c p) x -> p c x", p=128)

    iThv, iTlv = pcv(iTh, LN), pcv(iTl, LN)

    with tile.TileContext(nc) as tc:
        # --- persistent pool (lives through both phases) ---
        with tc.tile_pool(name="pers", bufs=1) as pers, \
             tc.tile_pool(name="dram", bufs=1, space="DRAM") as dram:
            kTh_sb = pers.tile([128, DC, LN], BF16)   # 32 KB/part
            kTl_sb = pers.tile([128, DC, LN], BF16)   # 32 KB/part
            alpha_ap = pers.tile([128, 1], F32)
            nc.vector.memset(alpha_ap, NEG_SLOPE)

            qTh_d = dram.tile([ST, 128, DC, 128], BF16)
            qTl_d = dram.tile([ST, 128, DC, 128], BF16)
            vls_sb = pers.tile([128, ST, D], BF16)   # 32 KB/part

            _psum_cm = tc.tile_pool(name="psum", bufs=1, space="PSUM")
            psum_pool = _psum_cm.__enter__()
            _ps_ctr = [0]

            def prep_psum(name):
                i_ = _ps_ctr[0] % 4
                _ps_ctr[0] += 1
                return psum_pool.tile([128, 512], F32, name=f"{name}{_ps_ctr[0]}", tag=f"sc{i_}")

            # ================= Phase A: projections =================
            with tc.tile_pool(name="pa_it", bufs=1) as pa_it, \
                 tc.tile_pool(name="pa_w", bufs=1) as pa_w, \
                 tc.tile_pool(name="pa_ev", bufs=1) as pa_ev:
                ith_sb = pa_it.tile([128, DC, LN], BF16)
                itl_sb = pa_it.tile([128, DC, LN], BF16)

                def load_w(hi_t, lo_t, chunked=False):
                    wh_sb = pa_w.tile([128, DC, D], BF16, name="wh_sb", tag="wh")
                    wl_sb = pa_w.tile([128, DC, D], BF16, name="wl_sb", tag="wl")
                    if chunked:
                        for dc in range(DC):
                            nc.sync.dma_start(out=wh_sb[:, dc], in_=pcv(hi_t, D)[:, dc])
                            nc.sync.dma_start(out=wl_sb[:, dc], in_=pcv(lo_t, D)[:, dc])
                    else:
                        nc.sync.dma_start(out=wh_sb, in_=pcv(hi_t, D))
                        nc.sync.dma_start(out=wl_sb, in_=pcv(lo_t, D))
                    return wh_sb, wl_sb

                # interleave per-dc chunks so dc=0 deps resolve early
                wq_h = pa_w.tile([128, DC, D], BF16, name="wh_sb", tag="wh")
                wq_l = pa_w.tile([128, DC, D], BF16, name="wl_sb", tag="wl")
                for dc in range(DC):
                    nc.sync.dma_start(out=wq_h[:, dc], in_=pcv(qh, D)[:, dc])
                    nc.sync.dma_start(out=ith_sb[:, dc], in_=iThv[:, dc])
                    nc.sync.dma_start(out=wq_l[:, dc], in_=pcv(ql, D)[:, dc])
                    nc.sync.dma_start(out=itl_sb[:, dc], in_=iTlv[:, dc])
                _wq = (wq_h, wq_l)

                # --- qT[h, s] (3-pass split) -> DRAM hi/lo ---
                wh_sb, wl_sb = _wq
                for hc in range(DC):
                    evh = pa_ev.tile([128, TB, 512], BF16, name="evh", tag="evh")
                    evl = pa_ev.tile([128, TB, 512], BF16, name="evl", tag="evl")
                    for sb in range(TB):
                        ps = prep_psum("ps_prep")
                        s_sl = slice(sb * 512, sb * 512 + 512)
                        for dc in range(DC):
                            first = dc == 0
                            last = dc == DC - 1
                            lw = wh_sb[:, dc, hc * 128:hc * 128 + 128]
                            ll = wl_sb[:, dc, hc * 128:hc * 128 + 128]
                            nc.tensor.matmul(ps, lw, ith_sb[:, dc, s_sl], start=first, stop=False)
                            nc.tensor.matmul(ps, lw, itl_sb[:, dc, s_sl], start=False, stop=False)
                            nc.tensor.matmul(ps, ll, ith_sb[:, dc, s_sl], start=False, stop=last)
                        nc.vector.tensor_copy(evh[:, sb], ps)
                        nc.vector.tensor_sub(evl[:, sb], ps, evh[:, sb])
                    evh_v = evh.rearrange("p b (si x) -> p (b si) x", x=128)
                    evl_v = evl.rearrange("p b (si x) -> p (b si) x", x=128)
                    qTh_dv = qTh_d.rearrange("si p c x -> p si c x")[:, :, hc, :]
                    qTl_dv = qTl_d.rearrange("si p c x -> p si c x")[:, :, hc, :]
                    nc.sync.dma_start(out=qTh_dv, in_=evh_v)
                    nc.sync.dma_start(out=qTl_dv, in_=evl_v)

                # --- kT[h, t] (3-pass split) -> resident SBUF hi/lo ---
                wh_sb, wl_sb = load_w(kh, kl, chunked=True)
                for hc in range(DC):
                    for tb in range(TB):
                        ps = prep_psum("ps_prep3")
                        t_sl = slice(tb * 512, tb * 512 + 512)
                        for dc in range(DC):
                            first = dc == 0
                            last = dc == DC - 1
                            lw = wh_sb[:, dc, hc * 128:hc * 128 + 128]
                            ll = wl_sb[:, dc, hc * 128:hc * 128 + 128]
                            nc.tensor.matmul(ps, lw, ith_sb[:, dc, t_sl], start=first, stop=False)
                            nc.tensor.matmul(ps, lw, itl_sb[:, dc, t_sl], start=False, stop=False)
                            nc.tensor.matmul(ps, ll, ith_sb[:, dc, t_sl], start=False, stop=last)
                        nc.vector.tensor_copy(kTh_sb[:, hc, t_sl], ps)
                        nc.vector.tensor_sub(kTl_sb[:, hc, t_sl], ps, kTh_sb[:, hc, t_sl])

# --- vls[t, e] (2-pass: iT full x vh, iTh x vl) -> DRAM ---
                wh_sb, wl_sb = load_w(vh, vl, chunked=True)
                for tc_ in range(ST):
                    t_sl = slice(tc_ * 128, tc_ * 128 + 128)
                    for eb in range(2):
                        ps = prep_psum("ps_prep2")
                        e_sl = slice(eb * 512, eb * 512 + 512)
                        for dc in range(DC):
                            first = dc == 0
                            last = dc == DC - 1
                            nc.tensor.matmul(ps, ith_sb[:, dc, t_sl], wh_sb[:, dc, e_sl], start=first, stop=False)
                            nc.tensor.matmul(ps, ith_sb[:, dc, t_sl], wl_sb[:, dc, e_sl], start=False, stop=last)
                        nc.vector.tensor_copy(vls_sb[:, tc_, e_sl], ps)

                            # ================= Phase B: attention + MLP =================
            with tc.tile_pool(name="pb_big", bufs=1) as pb_big, \
                 tc.tile_pool(name="pb_str", bufs=2) as pb_str, \
                 tc.tile_pool(name="pb_att", bufs=2) as pb_att, \
                 tc.tile_pool(name="pb_one", bufs=1) as pb_one, \
                 tc.tile_pool(name="pb_st", bufs=2) as pb_st:
                pb_sc = psum_pool
                pb_mm = psum_pool
                mlp_sb = pb_big.tile([128, DC, D], BF16)    # 16 KB/part
                nc.sync.dma_start(out=mlp_sb, in_=pcv(mlpb, D))

                for g in range(4):        # s-groups of 512
                    gs = slice(g * 512, g * 512 + 512)
                    attT_t = pb_one.tile([128, ST, 512], BF16, name="attT", tag="attT")   # 16 KB
                    itg_t = pb_one.tile([128, DC, 512], BF16, name="itg", tag="itg")      # 8 KB
                    ret_t = pb_one.tile([128, DC, 512], BF16, name="ret", tag="ret", bufs=2)  # 8 KB x2
                    nc.sync.dma_start(out=itg_t, in_=iThv[:, :, gs])

                    for st4 in range(4):
                        si = g * 4 + st4
                        s_sl = slice(si * 128, si * 128 + 128)
                        qtsh = pb_str.tile([128, DC, 128], BF16, name="qtsh", tag="qtsh")
                        qtsl = pb_str.tile([128, DC, 128], BF16, name="qtsl", tag="qtsl")
                        nc.sync.dma_start(out=qtsh, in_=qTh_d[si])
                        nc.sync.dma_start(out=qtsl, in_=qTl_d[si])

                        scs = [
                            pb_sc.tile([128, 512], F32, name=f"sc{tb}", tag=f"sc{tb}")
                            for tb in range(TB)
                        ]
                        for hc in range(DC):
                            first = hc == 0
                            last = hc == DC - 1
                            for tb in range(TB):
                                t_sl = slice(tb * 512, tb * 512 + 512)
                                nc.tensor.matmul(scs[tb], qtsh[:, hc], kTh_sb[:, hc, t_sl], start=first, stop=False)
                                nc.tensor.matmul(scs[tb], qtsh[:, hc], kTl_sb[:, hc, t_sl], start=False, stop=False)
                                nc.tensor.matmul(scs[tb], qtsl[:, hc], kTh_sb[:, hc, t_sl], start=False, stop=last)

                        # Per-t-block softmax: local max + exp immediately
                        # (frees each PSUM bank early), then algebraic
                        # rescale by f_tb = e^(m_tb - M) / S.
                        st_t = pb_st.tile([128, 24], F32, name="st_t", tag="stats")
                        negm4 = st_t[:, 0:4]
                        sums = st_t[:, 4:8]
                        negM = st_t[:, 8:9]
                        S = st_t[:, 9:10]
                        recip = st_t[:, 10:11]
                        g4 = st_t[:, 12:16]
                        f4 = st_t[:, 16:20]
                        gs = st_t[:, 20:24]
                        att32 = pb_att.tile([128, LN], F32, name="att32", tag="att32", bufs=1)
                        for tb in range(TB):
                            nc.vector.reduce_max(negm4[:, tb:tb + 1], scs[tb], axis=Axis.X, negate=True)
                            nc.scalar.activation(
                                out=att32[:, tb * 512:tb * 512 + 512], in_=scs[tb],
                                func=Act.Exp, bias=negm4[:, tb:tb + 1], scale=1.0,
                                accum_out=sums[:, tb:tb + 1],
                            )
                        nc.vector.tensor_reduce(negM, negm4, axis=Axis.X, op=mybir.AluOpType.min)
                        nc.scalar.activation(out=g4, in_=negm4, func=Act.Exp, bias=negM, scale=-1.0)
                        nc.vector.tensor_mul(gs, g4, sums)
                        nc.vector.reduce_sum(S, gs, axis=Axis.X)
                        nc.vector.reciprocal(recip, S)
                        nc.vector.tensor_scalar_mul(f4, g4, recip)

                        att_t = pb_att.tile([128, LN], BF16, name="att_t", tag="att")
                        for tb in range(TB):
                            nc.vector.tensor_scalar_mul(
                                att_t[:, tb * 512:tb * 512 + 512],
                                att32[:, tb * 512:tb * 512 + 512],
                                f4[:, tb:tb + 1],
                            )
                        nc.sync.dma_start_transpose(
                            out=attT_t[:, :, st4 * 128:st4 * 128 + 128], in_=att_t
                        )

                    # att @ vls (+ residual) -> retT[e, s-block]
                    for ec in range(DC):
                        psa = pb_mm.tile([128, 512], F32, name="psa", tag="av", bufs=2)
                        for tc_ in range(ST):
                            nc.tensor.matmul(
                                psa,
                                vls_sb[:, tc_, ec * 128:ec * 128 + 128],
                                attT_t[:, tc_, :],
                                start=(tc_ == 0), stop=(tc_ == ST - 1),
                            )
                        nc.vector.tensor_add(ret_t[:, ec, :], psa, itg_t[:, ec, :])

                    # (ret @ mlp) -> leaky relu -> + bias -> out
                    for st4 in range(4):
                        si = g * 4 + st4
                        s_sl = slice(si * 128, si * 128 + 128)
                        bias_t = pb_str.tile([128, D], F32, name="bias_t", tag="bias")
                        nc.sync.dma_start(out=bias_t, in_=bias.ap()[s_sl, :])
                        out_t = pb_str.tile([128, D], F32, name="out_t", tag="out")
                        for ob in range(2):
                            pso = pb_mm.tile([128, 512], F32, name="pso", tag="om", bufs=2)
                            o_sl = slice(ob * 512, ob * 512 + 512)
                            for ec in range(DC):
                                nc.tensor.matmul(
                                    pso,
                                    ret_t[:, ec, st4 * 128:st4 * 128 + 128],
                                    mlp_sb[:, ec, o_sl],
                                    start=(ec == 0), stop=(ec == DC - 1),
                                )
                            nc.scalar.activation(
                                out=out_t[:, o_sl], in_=pso, func=Act.Prelu,
                                bias=0.0, scale=1.0, alpha=alpha_ap,
                            )
                        nc.vector.tensor_add(out_t, out_t, bias_t)
                        nc.sync.dma_start(out=out_d.ap()[s_sl, :], in_=out_t)

            _psum_cm.__exit__(None, None, None)

    nc.compile()
    return nc


def _get_nc():
    global _cached_nc
    if _cached_nc is None:
        _cached_nc = _build()
    return _cached_nc


def _split_bf16(x):
    hi = x.astype(ml_dtypes.bfloat16)
    lo = (x - hi.astype(np.float32)).astype(ml_dtypes.bfloat16)
    return hi, lo


def kernel(i, k, q, v, mlp, bias):
    i = np.asarray(i, dtype=np.float32)
    k = np.asarray(k, dtype=np.float32)
    q = np.asarray(q, dtype=np.float32)
    v = np.asarray(v, dtype=np.float32)
    mlp = np.asarray(mlp, dtype=np.float32)
    bias = np.asarray(bias, dtype=np.float32)

    qh, ql = _split_bf16(q)
    kh, kl = _split_bf16(k)
    vh, vl = _split_bf16(v)
    mlpb = mlp.astype(ml_dtypes.bfloat16)

    shared = dict(qh=qh, ql=ql, kh=kh, kl=kl, vh=vh, vl=vl, mlpb=mlpb, bias=bias)
    in_maps = []
    for b in range(N_CORES):
        iT = np.ascontiguousarray(i[b].T)
        iTh, iTl = _split_bf16(iT)
        in_maps.append(dict(iTh=iTh, iTl=iTl, **shared))

    nc = _get_nc()
    res = bass_utils.run_bass_kernel_spmd(nc, in_maps, core_ids=list(range(N_CORES)))
    return np.stack([res.results[b]["out"] for b in range(N_CORES)])


# revision 11
# speedup vs baseline: 1.4503x; 1.0028x over previous
"""Trainium2 Bass kernel for nn_AttentionBlock (B=8, LN=2048, IDM=HDM=ODM=1024).

Sharding: data-parallel over batch, one batch element per NeuronCore (8 cores).

Per-core computation (batch element b):
    queries = i @ q ; keys = i @ k                    [ln, hdm]
    scores  = queries @ keys.T                        [ln, ln]
    att     = softmax(scores, axis=-1)
    vls     = i @ v                                   [ln, idm]
    ret     = att @ vls + i
    out     = leaky_relu(ret @ mlp, 0.2) + bias

Precision strategy: the softmax exponent amplifies matmul operand
rounding, so the Q/K path (q/k projections and scores) uses 3-pass
bf16 split matmuls (hi/lo decomposition, ~fp32 quality). The value/MLP
path tolerates bf16. All accumulation is fp32 in PSUM.

Layout strategy: everything is computed with the contraction dim on
partitions. The host pre-transposes i (iT = i.T) and pre-splits
operands into bf16 hi/lo pairs; on-chip phases:
  A) kT/qT/vls projections (qT, vls staged via DRAM),
  B) per 128-row s-tile: scores -> softmax -> DMA-transpose(att) ->
     att @ vls (+residual) -> @ mlp -> leaky-relu + bias.
"""
import os
import numpy as np
import ml_dtypes

import concourse.bacc as bacc
import concourse.mybir as mybir
import concourse.tile as tile
from concourse import bass_utils

F32 = mybir.dt.float32
BF16 = mybir.dt.bfloat16
Act = mybir.ActivationFunctionType
Axis = mybir.AxisListType

LN = 2048      # sequence length
D = 1024       # idm = hdm = odm
N_CORES = 8
DC = D // 128      # 8 contraction chunks
ST = LN // 128     # 16 s-tiles
TB = LN // 512     # 4 t-blocks (N=512)
NEG_SLOPE = 0.2

_cached_nc = None


def _build():
    nc = bacc.Bacc("TRN2", target_bir_lowering=False, debug=False)

    # Inputs (per core): host provides iT (= i_b.T) and all weights as
    # bf16 hi/lo splits. bias stays fp32.
    iTh = nc.dram_tensor("iTh", [D, LN], BF16, kind="ExternalInput")
    iTl = nc.dram_tensor("iTl", [D, LN], BF16, kind="ExternalInput")
    qh = nc.dram_tensor("qh", [D, D], BF16, kind="ExternalInput")
    ql = nc.dram_tensor("ql", [D, D], BF16, kind="ExternalInput")
    kh = nc.dram_tensor("kh", [D, D], BF16, kind="ExternalInput")
    kl = nc.dram_tensor("kl", [D, D], BF16, kind="ExternalInput")
    vh = nc.dram_tensor("vh", [D, D], BF16, kind="ExternalInput")
    vl = nc.dram_tensor("vl", [D, D], BF16, kind="ExternalInput")
    mlpb = nc.dram_tensor("mlpb", [D, D], BF16, kind="ExternalInput")
    bias = nc.dram_tensor("bias", [LN, D], F32, kind="ExternalInput")
    out_d = nc.dram_tensor("out", [LN, D], F32, kind="ExternalOutput")

    # [D, X] viewed as [128 partitions, DC chunks, X]
    def pcv(t, x):
        return t.ap().rearrange("(c p) x -> p c x", p=128)

    iThv, iTlv = pcv(iTh, LN), pcv(iTl, LN)

    with tile.TileContext(nc) as tc:
        # --- persistent pool (lives through both phases) ---
        with tc.tile_pool(name="pers", bufs=1) as pers, \
             tc.tile_pool(name="dram", bufs=1, space="DRAM") as dram:
            kTh_sb = pers.tile([128, DC, LN], BF16)   # 32 KB/part
            kTl_sb = pers.tile([128, DC, LN], BF16)   # 32 KB/part
            alpha_ap = pers.tile([128, 1], F32)
            nc.vector.memset(alpha_ap, NEG_SLOPE)

            qTh_d = dram.tile([ST, 128, DC, 128], BF16)
            qTl_d = dram.tile([ST, 128, DC, 128], BF16)
            vls_sb = pers.tile([128, ST, D], BF16)   # 32 KB/part

            _psum_cm = tc.tile_pool(name="psum", bufs=1, space="PSUM")
            psum_pool = _psum_cm.__enter__()
            _ps_ctr = [0]

            def prep_psum(name):
                i_ = _ps_ctr[0] % 4
                _ps_ctr[0] += 1
                return psum_pool.tile([128, 512], F32, name=f"{name}{_ps_ctr[0]}", tag=f"sc{i_}")

            # ================= Phase A: projections =================
            with tc.tile_pool(name="pa_it", bufs=1) as pa_it, \
                 tc.tile_pool(name="pa_w", bufs=1) as pa_w, \
                 tc.tile_pool(name="pa_ev", bufs=1) as pa_ev:
                ith_sb = pa_it.tile([128, DC, LN], BF16)
                itl_sb = pa_it.tile([128, DC, LN], BF16)

                def load_w(hi_t, lo_t, chunked=False):
                    wh_sb = pa_w.tile([128, DC, D], BF16, name="wh_sb", tag="wh")
                    wl_sb = pa_w.tile([128, DC, D], BF16, name="wl_sb", tag="wl")
                    if chunked:
                        for dc in range(DC):
                            nc.sync.dma_start(out=wh_sb[:, dc], in_=pcv(hi_t, D)[:, dc])
                            nc.sync.dma_start(out=wl_sb[:, dc], in_=pcv(lo_t, D)[:, dc])
                    else:
                        nc.sync.dma_start(out=wh_sb, in_=pcv(hi_t, D))
                        nc.sync.dma_start(out=wl_sb, in_=pcv(lo_t, D))
                    return wh_sb, wl_sb

                # interleave per-dc chunks so dc=0 deps resolve early
                wq_h = pa_w.tile([128, DC, D], BF16, name="wh_sb", tag="wh")
                wq_l = pa_w.tile([128, DC, D], BF16, name="wl_sb", tag="wl")
                for dc in range(DC):
                    nc.sync.dma_start(out=wq_h[:, dc], in_=pcv(qh, D)[:, dc])
                    nc.sync.dma_start(out=ith_sb[:, dc], in_=iThv[:, dc])
                    nc.sync.dma_start(out=wq_l[:, dc], in_=pcv(ql, D)[:, dc])
                    nc.sync.dma_start(out=itl_sb[:, dc], in_=iTlv[:, dc])
                _wq = (wq_h, wq_l)

                # --- qT[h, s] (3-pass split) -> DRAM hi/lo ---
                wh_sb, wl_sb = _wq
                for hc in range(DC):
                    evh = pa_ev.tile([128, TB, 512], BF16, name="evh", tag="evh")
                    evl = pa_ev.tile([128, TB, 512], BF16, name="evl", tag="evl")
                    for sb in range(TB):
                        ps = prep_psum("ps_prep")
                        s_sl = slice(sb * 512, sb * 512 + 512)
                        for dc in range(DC):
                            first = dc == 0
                            last = dc == DC - 1
                            lw = wh_sb[:, dc, hc * 128:hc * 128 + 128]
                            ll = wl_sb[:, dc, hc * 128:hc * 128 + 128]
                            nc.tensor.matmul(ps, lw, ith_sb[:, dc, s_sl], start=first, stop=False)
                            nc.tensor.matmul(ps, lw, itl_sb[:, dc, s_sl], start=False, stop=False)
                            nc.tensor.matmul(ps, ll, ith_sb[:, dc, s_sl], start=False, stop=last)
                        nc.vector.tensor_copy(evh[:, sb], ps)
                        nc.vector.tensor_sub(evl[:, sb], ps, evh[:, sb])
                    evh_v = evh.rearrange("p b (si x) -> p (b si) x", x=128)
                    evl_v = evl.rearrange("p b (si x) -> p (b si) x", x=128)
                    qTh_dv = qTh_d.rearrange("si p c x -> p si c x")[:, :, hc, :]
                    qTl_dv = qTl_d.rearrange("si p c x -> p si c x")[:, :, hc, :]
                    nc.sync.dma_start(out=qTh_dv, in_=evh_v)
                    nc.sync.dma_start(out=qTl_dv, in_=evl_v)

                # --- kT[h, t] (3-pass split) -> resident SBUF hi/lo ---
                wh_sb, wl_sb = load_w(kh, kl, chunked=True)
                for hc in range(DC):
                    for tb in range(TB):
                        ps = prep_psum("ps_prep3")
                        t_sl = slice(tb * 512, tb * 512 + 512)
                        for dc in range(DC):
                            first = dc == 0
                            last = dc == DC - 1
                            lw = wh_sb[:, dc, hc * 128:hc * 128 + 128]
                            ll = wl_sb[:, dc, hc * 128:hc * 128 + 128]
                            nc.tensor.matmul(ps, lw, ith_sb[:, dc, t_sl], start=first, stop=False)
                            nc.tensor.matmul(ps, lw, itl_sb[:, dc, t_sl], start=False, stop=False)
                            nc.tensor.matmul(ps, ll, ith_sb[:, dc, t_sl], start=False, stop=last)
                        nc.vector.tensor_copy(kTh_sb[:, hc, t_sl], ps)
                        nc.vector.tensor_sub(kTl_sb[:, hc, t_sl], ps, kTh_sb[:, hc, t_sl])

# --- vls[t, e] (2-pass: iT full x vh, iTh x vl) -> DRAM ---
                wh_sb, wl_sb = load_w(vh, vl, chunked=True)
                for tc_ in range(ST):
                    t_sl = slice(tc_ * 128, tc_ * 128 + 128)
                    for eb in range(2):
                        ps = prep_psum("ps_prep2")
                        e_sl = slice(eb * 512, eb * 512 + 512)
                        for dc in range(DC):
                            first = dc == 0
                            last = dc == DC - 1
                            nc.tensor.matmul(ps, ith_sb[:, dc, t_sl], wh_sb[:, dc, e_sl], start=first, stop=False)
                            nc.tensor.matmul(ps, ith_sb[:, dc, t_sl], wl_sb[:, dc, e_sl], start=False, stop=last)
                        nc.vector.tensor_copy(vls_sb[:, tc_, e_sl], ps)

                            # ================= Phase B: attention + MLP =================
            with tc.tile_pool(name="pb_big", bufs=1) as pb_big, \
                 tc.tile_pool(name="pb_str", bufs=2) as pb_str, \
                 tc.tile_pool(name="pb_att", bufs=2) as pb_att, \
                 tc.tile_pool(name="pb_one", bufs=1) as pb_one, \
                 tc.tile_pool(name="pb_st", bufs=2) as pb_st:
                pb_sc = psum_pool
                pb_mm = psum_pool
                mlp_sb = pb_big.tile([128, DC, D], BF16)    # 16 KB/part
                mlp_loaded = [False]

                for g in range(4):        # s-groups of 512
                    gs = slice(g * 512, g * 512 + 512)
                    attT_t = pb_one.tile([128, ST, 512], BF16, name="attT", tag="attT")   # 16 KB
                    itg_t = pb_one.tile([128, DC, 512], BF16, name="itg", tag="itg")      # 8 KB
                    ret_t = pb_one.tile([128, DC, 512], BF16, name="ret", tag="ret", bufs=2)  # 8 KB x2

                    for st4 in range(4):
                        si = g * 4 + st4
                        s_sl = slice(si * 128, si * 128 + 128)
                        qtsh = pb_str.tile([128, DC, 128], BF16, name="qtsh", tag="qtsh")
                        qtsl = pb_str.tile([128, DC, 128], BF16, name="qtsl", tag="qtsl")
                        nc.sync.dma_start(out=qtsh, in_=qTh_d[si])
                        nc.sync.dma_start(out=qtsl, in_=qTl_d[si])

                        scs = [
                            pb_sc.tile([128, 512], F32, name=f"sc{tb}", tag=f"sc{tb}")
                            for tb in range(TB)
                        ]
                        for hc in range(DC):
                            first = hc == 0
                            last = hc == DC - 1
                            for tb in range(TB):
                                t_sl = slice(tb * 512, tb * 512 + 512)
                                nc.tensor.matmul(scs[tb], qtsh[:, hc], kTh_sb[:, hc, t_sl], start=first, stop=False)
                                nc.tensor.matmul(scs[tb], qtsh[:, hc], kTl_sb[:, hc, t_sl], start=False, stop=False)
                                nc.tensor.matmul(scs[tb], qtsl[:, hc], kTh_sb[:, hc, t_sl], start=False, stop=last)

                        # Per-t-block softmax: local max + exp immediately
                        # (frees each PSUM bank early), then algebraic
                        # rescale by f_tb = e^(m_tb - M) / S.
                        st_t = pb_st.tile([128, 24], F32, name="st_t", tag="stats")
                        negm4 = st_t[:, 0:4]
                        sums = st_t[:, 4:8]
                        negM = st_t[:, 8:9]
                        S = st_t[:, 9:10]
                        recip = st_t[:, 10:11]
                        g4 = st_t[:, 12:16]
                        f4 = st_t[:, 16:20]
                        gs4 = st_t[:, 20:24]
                        att32 = pb_att.tile([128, LN], F32, name="att32", tag="att32", bufs=1)
                        for tb in range(TB):
                            nc.vector.reduce_max(negm4[:, tb:tb + 1], scs[tb], axis=Axis.X, negate=True)
                            nc.scalar.activation(
                                out=att32[:, tb * 512:tb * 512 + 512], in_=scs[tb],
                                func=Act.Exp, bias=negm4[:, tb:tb + 1], scale=1.0,
                                accum_out=sums[:, tb:tb + 1],
                            )
                        nc.vector.tensor_reduce(negM, negm4, axis=Axis.X, op=mybir.AluOpType.min)
                        nc.scalar.activation(out=g4, in_=negm4, func=Act.Exp, bias=negM, scale=-1.0)
                        nc.vector.tensor_mul(gs4, g4, sums)
                        nc.vector.reduce_sum(S, gs4, axis=Axis.X)
                        nc.vector.reciprocal(recip, S)
                        nc.vector.tensor_scalar_mul(f4, g4, recip)

                        att_t = pb_att.tile([128, LN], BF16, name="att_t", tag="att")
                        for tb in range(TB):
                            nc.vector.tensor_scalar_mul(
                                att_t[:, tb * 512:tb * 512 + 512],
                                att32[:, tb * 512:tb * 512 + 512],
                                f4[:, tb:tb + 1],
                            )
                        nc.sync.dma_start_transpose(
                            out=attT_t[:, :, st4 * 128:st4 * 128 + 128], in_=att_t
                        )

                    # att @ vls (+ residual) -> retT[e, s-block]
                    nc.sync.dma_start(out=itg_t, in_=iThv[:, :, gs])
                    if not mlp_loaded[0]:
                        nc.sync.dma_start(out=mlp_sb, in_=pcv(mlpb, D))
                        mlp_loaded[0] = True
                    for ec in range(DC):
                        psa = pb_mm.tile([128, 512], F32, name="psa", tag="av", bufs=2)
                        for tc_ in range(ST):
                            nc.tensor.matmul(
                                psa,
                                vls_sb[:, tc_, ec * 128:ec * 128 + 128],
                                attT_t[:, tc_, :],
                                start=(tc_ == 0), stop=(tc_ == ST - 1),
                            )
                        nc.vector.tensor_add(ret_t[:, ec, :], psa, itg_t[:, ec, :])

                    # (ret @ mlp) -> leaky relu -> + bias -> out
                    for st4 in range(4):
                        si = g * 4 + st4
                        s_sl = slice(si * 128, si * 128 + 128)
                        bias_t = pb_str.tile([128, D], F32, name="bias_t", tag="bias")
                        nc.sync.dma_start(out=bias_t, in_=bias.ap()[s_sl, :])
                        out_t = pb_str.tile([128, D], F32, name="out_t", tag="out")
                        for ob in range(2):
                            pso = pb_mm.tile([128, 512], F32, name="pso", tag="om", bufs=2)
                            o_sl = slice(ob * 512, ob * 512 + 512)
                            for ec in range(DC):
                                nc.tensor.matmul(
                                    pso,
                                    ret_t[:, ec, st4 * 128:st4 * 128 + 128],
                                    mlp_sb[:, ec, o_sl],
                                    start=(ec == 0), stop=(ec == DC - 1),
                                )
                            nc.scalar.activation(
                                out=out_t[:, o_sl], in_=pso, func=Act.Prelu,
                                bias=0.0, scale=1.0, alpha=alpha_ap,
                            )
                        nc.vector.tensor_add(out_t, out_t, bias_t)
                        nc.sync.dma_start(out=out_d.ap()[s_sl, :], in_=out_t)

            _psum_cm.__exit__(None, None, None)

    nc.compile()
    return nc


def _get_nc():
    global _cached_nc
    if _cached_nc is None:
        _cached_nc = _build()
    return _cached_nc


def _split_bf16(x):
    hi = x.astype(ml_dtypes.bfloat16)
    lo = (x - hi.astype(np.float32)).astype(ml_dtypes.bfloat16)
    return hi, lo


def kernel(i, k, q, v, mlp, bias):
    i = np.asarray(i, dtype=np.float32)
    k = np.asarray(k, dtype=np.float32)
    q = np.asarray(q, dtype=np.float32)
    v = np.asarray(v, dtype=np.float32)
    mlp = np.asarray(mlp, dtype=np.float32)
    bias = np.asarray(bias, dtype=np.float32)

    qh, ql = _split_bf16(q)
    kh, kl = _split_bf16(k)
    vh, vl = _split_bf16(v)
    mlpb = mlp.astype(ml_dtypes.bfloat16)

    shared = dict(qh=qh, ql=ql, kh=kh, kl=kl, vh=vh, vl=vl, mlpb=mlpb, bias=bias)
    in_maps = []
    for b in range(N_CORES):
        iT = np.ascontiguousarray(i[b].T)
        iTh, iTl = _split_bf16(iT)
        in_maps.append(dict(iTh=iTh, iTl=iTl, **shared))

    nc = _get_nc()
    res = bass_utils.run_bass_kernel_spmd(nc, in_maps, core_ids=list(range(N_CORES)))
    return np.stack([res.results[b]["out"] for b in range(N_CORES)])
